# revision 1
# baseline (speedup 1.0000x reference)
"""Decoder block (single-head causal attention + GELU FFN) on 8 TRN2 NeuronCores.

Sharding: pure data parallel, no collectives. Core c handles batch b = c//2 and
1024 query tokens of that batch, chosen as four 256-token chunks that balance
the causal-attention workload:
  even cores (half 0): chunks 0, 3, 4, 7
  odd  cores (half 1): chunks 1, 2, 5, 6
The slot pairing makes the static k-tile counts per slot (4, 8, 12, 16) cover
both cores' needs with minimal waste (ideal is (2..16); the gap is zeroed by
the data-driven qpos mask). The SPMD program is identical on every core; all
per-core differences are data (which tokens are in xq, qpos values that drive
on-chip causal-mask creation).

Performance structure:
  - every matmul operand is fp16 (same PE rate as fp32r, half the DMA/SBUF)
  - K and V projections fused into one pass over x (x read once)
  - K^T and V stay resident in SBUF (no DRAM round-trips)
  - DMA issue split across engines: weights on gpsimd, x on sync, stores on
    scalar, consts on scalar — so tile back-pressure in one stream never
    delays another
  - all long-lived tiles (incl. wq/wo/xq) live in one pool so their loads are
    not gated on earlier phases' SBUF frees
  - scalar engine runs only Identity in P1 and only Exp in P2 (activation
    table reloads cost 1.3us each)
"""

import numpy as np

D = 1024  # model dim
S = 2048  # sequence length
B = 4  # batch
M = 4096  # FFN dim
CH = 256  # q chunk (slot) size
NSLOT = 4  # q slots per core
NDT = D // 128  # 8 d-tiles
N_CORES = 8
NKT = [4, 8, 12, 16]  # k-tiles per slot (static max over the two paired cores)

_PROGRAM = None  # cached compiled program


def _build_program():
    import sys

    if "/opt/trn_rl_repo" not in sys.path:
        sys.path.insert(0, "/opt/trn_rl_repo")
    import concourse.bass as bass
    import concourse.tile as tile
    import concourse.mybir as mybir
    from concourse import bacc
    from concourse.bass import ts

    dt = mybir.dt
    AF = mybir.ActivationFunctionType
    ALU = mybir.AluOpType
    F32, F32R, F16 = dt.float32, dt.float32r, dt.float16

    nc = bacc.Bacc("TRN2", target_bir_lowering=False, debug=False, num_devices=8)

    # ---------------- DRAM I/O ----------------
    xT = nc.dram_tensor("xT", [D, S], F16, kind="ExternalInput").ap()
    xkT = nc.dram_tensor("xkT", [D, S // 2], F16, kind="ExternalInput").ap()
    xoT = nc.dram_tensor("xoT", [D, 4 * CH], F16, kind="ExternalInput").ap()
    wqT = nc.dram_tensor("wqT", [D, D], F16, kind="ExternalInput").ap()
    wkT = nc.dram_tensor("wkT", [D, D], F16, kind="ExternalInput").ap()
    wvT = nc.dram_tensor("wvT", [D, D], F16, kind="ExternalInput").ap()
    woT = nc.dram_tensor("woT", [D, D], F16, kind="ExternalInput").ap()
    wfT = nc.dram_tensor("wfT", [D, M], F16, kind="ExternalInput").ap()
    bq = nc.dram_tensor("bq", [128, D // 128], F32, kind="ExternalInput").ap()
    bk = nc.dram_tensor("bk", [128, D // 128], F32, kind="ExternalInput").ap()
    bo2 = nc.dram_tensor("bo2", [128, D // 128], F32, kind="ExternalInput").ap()
    bfT = nc.dram_tensor("bfT", [128, M // 128], F32, kind="ExternalInput").ap()
    qpos = nc.dram_tensor("qpos", [1, 4 * CH], F32R, kind="ExternalInput").ap()
    iota_kt = nc.dram_tensor("iota_kt", [128, S // 128], F32, kind="ExternalInput").ap()
    ffT = nc.dram_tensor("ffT", [M, 4 * CH], F16, kind="ExternalOutput").ap()

    with tile.TileContext(nc) as tc:
        with (
            tc.tile_pool(name="const", bufs=1) as cpool,
            tc.tile_pool(name="psum", bufs=1, space="PSUM") as pspool,
        ):
            # ---------------- constants (scalar engine issues these) --------
            ones_col_bf = cpool.tile([128, 1], F16, name="ones_col_bf", tag="ones_col_bf")
            nc.vector.memset(ones_col_bf[:], 1.0)
            ones_row_f = cpool.tile([1, 128], F32, name="ones_row_f", tag="ones_row_f")
            nc.vector.memset(ones_row_f[:], 1.0)
            ones_row = cpool.tile([1, 128], F32R, name="ones_row", tag="ones_row")
            nc.vector.tensor_copy(ones_row[:], ones_row_f[:])
            iota_sb = cpool.tile([128, S // 128], F32, name="iota", tag="iota")
            nc.scalar.dma_start(iota_sb[:], iota_kt[:])
            bq_sb = cpool.tile([128, D // 128], F32, name="bq", tag="bq")
            nc.scalar.dma_start(bq_sb[:], bq[:])
            bk_sb = cpool.tile([128, D // 128], F32, name="bk", tag="bk")
            nc.scalar.dma_start(bk_sb[:], bk[:])
            bo2_sb = cpool.tile([128, D // 128], F32, name="bo2", tag="bo2")
            nc.scalar.dma_start(bo2_sb[:], bo2[:])
            bf_sb = cpool.tile([128, M // 128], F32, name="bf", tag="bf")
            nc.scalar.dma_start(bf_sb[:], bfT[:])
            qpos_row = cpool.tile([1, 4 * CH], F32R, name="qpos_row", tag="qpos_row")
            nc.scalar.dma_start(qpos_row[:], qpos[:])

            # qposB is broadcast later (just before P2): putting its matmul
            # here would head-block the in-order PE stream on the qpos DMA
            qposB = cpool.tile([128, 4 * CH], F32, name="qposB", tag="qposB")

            # ------------- long-lived tiles: one pool spanning P1..P4 -------
            with (
                tc.tile_pool(name="main", bufs=1) as mp,
                tc.tile_pool(name="dram", bufs=1, space="DRAM") as dram,
            ):
                kT = [mp.tile([128, S], F16, name=f"kT{i}", tag=f"kT{i}") for i in range(NDT)]
                vt = [mp.tile([128, D], F16, name=f"v{k}", tag=f"v{k}") for k in range(16)]
                wq_sb = [mp.tile([128, D], F16, name=f"wq{i}", tag=f"wq{i}") for i in range(NDT)]
                wo_sb = [mp.tile([128, D], F16, name=f"wo{i}", tag=f"wo{i}") for i in range(NDT)]
                xq = [mp.tile([128, 4 * CH], F16, name=f"xq{i}", tag=f"xq{i}") for i in range(NDT)]
                qT = [
                    [mp.tile([128, 512], F16, name=f"qT{dt_}_{qb}", tag=f"qT{dt_}_{qb}") for qb in range(2)]
                    for dt_ in range(NDT)
                ]
                attnT = [
                    [mp.tile([128, 512], F16, name=f"at{dt_}_{qb}", tag=f"at{dt_}_{qb}") for qb in range(2)]
                    for dt_ in range(NDT)
                ]
                # DRAM bounce buffers for the pairwise K AllGather
                kv_in = dram.tile([D, S // 2], F16, name="kv_in", tag="kv_in")
                kv_out = dram.tile([2, D, S // 2], F16, name="kv_out", tag="kv_out")

                # ---------------- P1: K/V projections (fused x pass) --------
                with tc.tile_pool(name="p1a", bufs=1) as p1a:
                    wk_sb = [p1a.tile([128, D], F16, name=f"wk{i}", tag=f"wk{i}") for i in range(NDT)]
                    wv_sb = [p1a.tile([128, D], F16, name=f"wv{i}", tag=f"wv{i}") for i in range(NDT)]
                    # weight streams on gpsimd, independent of x on sync.
                    # Order = first-use order: wk h0 (og=0 groups), wk h1,
                    # wv, then wq / wo / xq (needed much later).
                    for h in range(2):
                        for i in range(NDT):
                            nc.gpsimd.dma_start(
                                wk_sb[i][:, ts(h, 512)], wkT[ts(i, 128), ts(h, 512)]
                            )
                    # wq right after wk: Q projection runs second (it fills
                    # the window while wv/xa still stream in)
                    for h in range(2):
                        for i in range(NDT):
                            nc.gpsimd.dma_start(
                                wq_sb[i][:, ts(h, 512)], wqT[ts(i, 128), ts(h, 512)]
                            )
                    xbufs = {}

                    def load_x(tb, src, xtag):
                        xblk = [p1a.tile([128, 512], F16, name=f"{xtag}{i}", tag=f"{xtag}{i}", bufs=2) for i in range(NDT)]
                        for i in range(NDT):
                            nc.sync.dma_start(xblk[i][:], src[ts(i, 128), ts(tb, 512)])
                        xbufs[(xtag, tb)] = xblk

                    def k_own_pass(tb):
                        # K projection over this core's OWN half of the
                        # sequence only; the peer half arrives via AllGather.
                        # Results stage into kT[:, 0:1024] (overwritten with
                        # the correctly-ordered gather output later).
                        # i-outer across 4 parallel PSUM banks, so the first
                        # matmul only needs wk[0] h0 + xblk[0]
                        xblk = xbufs.pop(("xa", tb))
                        for og in range(2):
                            ps4 = [
                                pspool.tile([128, 512], F32, name=f"at{j}", tag=f"at{j}", bufs=1)
                                for j in range(4)
                            ]
                            for i in range(NDT):
                                for j in range(4):
                                    nc.tensor.matmul(
                                        ps4[j][:],
                                        wk_sb[i][:, ts(og * 4 + j, 128)],
                                        xblk[i][:],
                                        start=(i == 0), stop=(i == NDT - 1),
                                    )
                            for j in range(4):
                                ot = og * 4 + j
                                nc.scalar.activation(
                                    kT[ot][:, ts(tb, 512)], ps4[j][:], AF.Identity,
                                    bias=bk_sb[:, ot : ot + 1],
                                )

                    def v_pass(tb):
                        # token-major, i-outer across 2 banks per tt
                        xblk = xbufs.pop(("xa", tb))
                        for tt in range(4):
                            ps2 = [
                                pspool.tile([128, 512], F32, name=f"mm{ob}", tag="mm", bufs=3)
                                for ob in range(2)
                            ]
                            for i in range(NDT):
                                for ob in range(2):
                                    nc.tensor.matmul(
                                        ps2[ob][:],
                                        xblk[i][:, ts(tt, 128)],
                                        wv_sb[i][:, ts(ob, 512)],
                                        start=(i == 0), stop=(i == NDT - 1),
                                    )
                            for ob in range(2):
                                nc.scalar.activation(
                                    vt[tb * 4 + tt][:, ts(ob, 512)], ps2[ob][:], AF.Identity
                                )

                    # K (own half) first; its gather runs while Q/V compute
                    load_x(0, xkT, "xa")
                    load_x(1, xkT, "xa")
                    # xq loads follow the xk stream on sync (Q runs second)
                    for i in range(NDT):
                        for h in range(2):
                            nc.sync.dma_start(
                                xq[i][:, ts(h, 512)], xoT[ts(i, 128), ts(h, 512)]
                            )
                    k_own_pass(0)
                    # qpos broadcast here: it fills part of the startup
                    # window where og1 waits on the wk h1 weight stream
                    for i in range(4 * CH // 512):
                        bc_ps = pspool.tile([128, 512], F32, name="small", tag="small", bufs=1)
                        nc.tensor.matmul(
                            bc_ps[:], ones_row[:], qpos_row[:, ts(i, 512)],
                            start=True, stop=True,
                        )
                        nc.scalar.activation(qposB[:, ts(i, 512)], bc_ps[:], AF.Identity)
                    k_own_pass(1)
                    # stage own K^T half to DRAM and gather the pair.
                    # Issued from sync (idle here): putting these on scalar
                    # delays the Q readouts behind them, which stalls the PE
                    # on PSUM-bank back-pressure (~4us measured)
                    for ot in range(NDT):
                        nc.sync.dma_start(kv_in[ts(ot, 128), :], kT[ot][:, 0 : S // 2])
                    nc.gpsimd.collective_compute(
                        "AllGather",
                        mybir.AluOpType.bypass,
                        replica_groups=[[0, 1], [2, 3], [4, 5], [6, 7]],
                        ins=[kv_in[:].opt()],
                        outs=[kv_out[:].opt()],
                    )
                    # wv / wo weight streams follow the trigger on gpsimd
                    for h in range(2):
                        for i in range(NDT):
                            nc.gpsimd.dma_start(
                                wv_sb[i][:, ts(h, 512)], wvT[ts(i, 128), ts(h, 512)]
                            )
                    for h in range(2):
                        for i in range(NDT):
                            nc.gpsimd.dma_start(
                                wo_sb[i][:, ts(h, 512)], woT[ts(i, 128), ts(h, 512)]
                            )

                    # ---------------- Q projection (fills the wv wait) ------
                    for qb in range(2):
                        for ot in range(NDT):
                            ps = pspool.tile([128, 512], F32, name="mm", tag="mm", bufs=3)
                            for i in range(NDT):
                                nc.tensor.matmul(
                                    ps[:], wq_sb[i][:, ts(ot, 128)], xq[i][:, ts(qb, 512)],
                                    start=(i == 0), stop=(i == NDT - 1),
                                )
                            nc.scalar.activation(
                                qT[ot][qb][:], ps[:], AF.Identity, bias=bq_sb[:, ot : ot + 1]
                            )

                    # V over the full sequence (still data-parallel-redundant)
                    load_x(0, xT, "xa")
                    load_x(1, xT, "xa")
                    v_pass(0)
                    load_x(2, xT, "xa")
                    v_pass(1)
                    load_x(3, xT, "xa")
                    v_pass(2)
                    v_pass(3)

                # gathered K^T readback in true token order (rank0 = tokens
                # 0:1024 first: attention slots 0/1 only need those)
                for r in range(2):
                    for i in range(NDT):
                        nc.sync.dma_start(
                            kT[i][:, r * (S // 2) : (r + 1) * (S // 2)],
                            kv_out[r, ts(i, 128), :],
                        )

                # ---------------- P2: attention (4 slots of 256 queries) ----
                with tc.tile_pool(name="p2", bufs=1) as p2:
                    for sl in range(NSLOT):
                        nkt = NKT[sl]
                        qmov = qT_slice = None  # doc: moving = qT[i][sl//2][:, (sl%2)*256:+256]
                        pt = [
                            p2.tile([128, CH], F16, name=f"pt{k}", tag=f"pt{k}", bufs=1)
                            for k in range(nkt)
                        ]
                        dn_ps = pspool.tile([1, CH], F32, name="small", tag="small", bufs=1)
                        for k in range(nkt):
                            ps = pspool.tile([128, CH], F32, name="mm", tag="mm", bufs=3)
                            for i in range(NDT):
                                nc.tensor.matmul(
                                    ps[:],
                                    kT[i][:, ts(k, 128)],
                                    qT[i][sl // 2][:, ts(sl % 2, CH)],
                                    start=(i == 0), stop=(i == NDT - 1),
                                )
                            masked = k >= nkt - 4
                            if masked:
                                praw = p2.tile([128, CH], F16, name="praw", tag="praw", bufs=2)
                                nc.scalar.activation(
                                    praw[:], ps[:], AF.Exp, scale=1.0 / 32.0
                                )
                                msk = p2.tile([128, CH], F16, name="msk", tag="msk", bufs=2)
                                nc.vector.tensor_scalar(
                                    out=msk[:],
                                    in0=qposB[:, ts(sl, CH)],
                                    scalar1=iota_sb[:, k : k + 1],
                                    scalar2=None,
                                    op0=ALU.is_ge,
                                )
                                nc.vector.tensor_tensor(
                                    out=pt[k][:], in0=praw[:], in1=msk[:],
                                    op=ALU.mult,
                                )
                            else:
                                nc.scalar.activation(
                                    pt[k][:], ps[:], AF.Exp, scale=1.0 / 32.0
                                )
                            nc.tensor.matmul(
                                dn_ps[:], ones_col_bf[:], pt[k][:],
                                start=(k == 0), stop=(k == nkt - 1),
                            )
                        # broadcast denom to 128 partitions, then reciprocal
                        # full-width (a [1,256] DVE op is ~10x slower than the
                        # [128,256] one)
                        dn_r = p2.tile([1, CH], F32R, name="dn_r", tag="dn_r", bufs=2)
                        nc.vector.tensor_copy(dn_r[:], dn_ps[:])
                        rb_ps = pspool.tile([128, CH], F32, name="small", tag="small", bufs=1)
                        nc.tensor.matmul(
                            rb_ps[:], ones_row[:], dn_r[:], start=True, stop=True
                        )
                        recipB = p2.tile([128, CH], F32, name="recipB", tag="recipB", bufs=1)
                        nc.vector.reciprocal(recipB[:], rb_ps[:])
                        # attn^T = (P @ V)^T scaled by 1/denom, two 4-bank passes
                        for half in range(2):
                            for d4 in range(4):
                                d_ = half * 4 + d4
                                aps = pspool.tile([128, CH], F32, name=f"at{d4}", tag=f"at{d4}", bufs=1)
                                for k in range(nkt):
                                    nc.tensor.matmul(
                                        aps[:],
                                        vt[k][:, ts(d_, 128)],
                                        pt[k][:],
                                        start=(k == 0), stop=(k == nkt - 1),
                                    )
                                nc.vector.tensor_tensor(
                                    out=attnT[d_][sl // 2][:, ts(sl % 2, CH)],
                                    in0=aps[:], in1=recipB[:],
                                    op=ALU.mult,
                                )

                # ---------------- P3 + P4 ----------------
                with tc.tile_pool(name="p34", bufs=1) as p34:
                    outT = [
                        [p34.tile([128, 512], F16, name=f"oT{dt_}_{qb}", tag=f"oT{dt_}_{qb}") for qb in range(2)]
                        for dt_ in range(NDT)
                    ]
                    # prefetch the first two FFN weight blocks during P3
                    wfb_pool = p34
                    for qb in range(2):
                        for ot in range(NDT):
                            ps = pspool.tile([128, 512], F32, name="mm", tag="mm", bufs=3)
                            for i in range(NDT):
                                nc.tensor.matmul(
                                    ps[:],
                                    wo_sb[i][:, ts(ot, 128)],
                                    attnT[i][qb][:],
                                    start=(i == 0), stop=(i == NDT - 1),
                                )
                            nc.scalar.activation(
                                outT[ot][qb][:], ps[:], AF.Identity, bias=bo2_sb[:, ot : ot + 1]
                            )

                    # ---------------- P4: FFN + GELU ----------------
                    for mb in range(M // 512):
                        wfb = [
                            wfb_pool.tile([128, 512], F16, name=f"wf{i}", tag=f"wf{i}", bufs=2)
                            for i in range(NDT)
                        ]
                        for i in range(NDT):
                            nc.gpsimd.dma_start(wfb[i][:], wfT[ts(i, 128), ts(mb, 512)])
                        for mt in range(4):
                            m = mb * 4 + mt
                            for qb in range(2):
                                ps = pspool.tile([128, 512], F32, name="mm", tag="mm", bufs=3)
                                for i in range(NDT):
                                    nc.tensor.matmul(
                                        ps[:],
                                        wfb[i][:, ts(mt, 128)],
                                        outT[i][qb][:],
                                        start=(i == 0), stop=(i == NDT - 1),
                                    )
                                st = p34.tile([128, 512], F16, name="ffstage", tag="ffstage", bufs=4)
                                nc.scalar.activation(
                                    st[:], ps[:], AF.Gelu, bias=bf_sb[:, m : m + 1]
                                )
                                # store issued by scalar right after its GELU
                                nc.scalar.dma_start(ffT[ts(m, 128), ts(qb, 512)], st[:])

    nc.compile()
    return nc


def _get_program():
    global _PROGRAM
    if _PROGRAM is None:
        _PROGRAM = _build_program()
    return _PROGRAM


def _owned_chunks(core):
    """The four 256-token chunk indices this core owns, in slot order."""
    if core % 2 == 0:
        return (0, 3, 4, 7)
    return (1, 2, 5, 6)


def _make_in_maps(x, Wq, bq, Wk, bk, Wv, bv, Wo, bo, Wf, bf):
    f32, f16 = np.float32, np.float16
    wqT = np.ascontiguousarray(Wq.T, dtype=f16)
    wkT = np.ascontiguousarray(Wk.T, dtype=f16)
    wvT = np.ascontiguousarray(Wv.T, dtype=f16)
    woT = np.ascontiguousarray(Wo.T, dtype=f16)
    wfT = np.ascontiguousarray(Wf.T, dtype=f16)
    bo2 = (Wo.astype(np.float64) @ bv.astype(np.float64) + bo.astype(np.float64))
    bo2 = np.ascontiguousarray(bo2.astype(f32).reshape(D // 128, 128).T)
    bfT = np.ascontiguousarray(bf.reshape(M // 128, 128).T, dtype=f32)
    iota = (
        np.arange(128, dtype=f32)[:, None]
        + 128.0 * np.arange(S // 128, dtype=f32)[None, :]
    )
    shared = {
        "wqT": wqT, "wkT": wkT, "wvT": wvT, "woT": woT, "wfT": wfT,
        "bq": np.ascontiguousarray(bq.reshape(D // 128, 128).T, dtype=f32),
        "bk": np.ascontiguousarray(bk.reshape(D // 128, 128).T, dtype=f32),
        "bo2": bo2,
        "bfT": bfT,
        "iota_kt": np.ascontiguousarray(iota),
    }
    in_maps = []
    for core in range(N_CORES):
        b = core // 2
        chunks = _owned_chunks(core)
        xTb = np.ascontiguousarray(x[b].T, dtype=f16)  # [D, S]
        half = core % 2  # rank within the pair: rank0 owns tokens 0:S/2
        xkT = np.ascontiguousarray(xTb[:, half * (S // 2) : (half + 1) * (S // 2)])
        xoT = np.ascontiguousarray(
            np.concatenate([xTb[:, c * CH : (c + 1) * CH] for c in chunks], axis=1)
        )
        qp = np.concatenate(
            [np.arange(c * CH, (c + 1) * CH) for c in chunks]
        ).astype(f32)[None, :]
        in_maps.append(
            {**shared, "xT": xTb, "xkT": xkT, "xoT": xoT,
             "qpos": np.ascontiguousarray(qp)}
        )
    return in_maps


def _run(inputs, trace=False, trace_cores=None, tmpdir=None):
    import sys

    if "/opt/trn_rl_repo" not in sys.path:
        sys.path.insert(0, "/opt/trn_rl_repo")
    from concourse.bass_utils import run_bass_kernel_spmd

    nc = _get_program()
    in_maps = _make_in_maps(**inputs)
    res = run_bass_kernel_spmd(
        nc, in_maps, list(range(N_CORES)), trace=trace, trace_cores=trace_cores,
        tmpdir=tmpdir,
    )
    out = np.empty((B, S, M), dtype=np.float32)
    for core in range(N_CORES):
        b = core // 2
        chunks = _owned_chunks(core)
        ffT = res.results[core]["ffT"]  # [M, 1024] fp16
        for sl, c in enumerate(chunks):
            out[b, c * CH : (c + 1) * CH] = (
                ffT[:, sl * CH : (sl + 1) * CH].T.astype(np.float32)
            )
    return out, res


def kernel(**inputs):
    out, _ = _run(inputs)
    return out



# revision 4
# speedup vs baseline: 1.0334x; 1.0334x over previous
"""Decoder block (single-head causal attention + GELU FFN) on 8 TRN2 NeuronCores.

Sharding: data parallel over batch (2 cores per batch), with the K AND V
projections token-split across the pair (each core projects its own half of
the sequence, then a pairwise AllGather shares both). Core c handles batch
b = c//2 and 1024 query tokens of that batch, chosen as four 256-token chunks
that balance the causal-attention workload:
  even cores (half 0): chunks 0, 3, 4, 7
  odd  cores (half 1): chunks 1, 2, 5, 6
The slot pairing makes the static k-tile counts per slot (4, 8, 12, 16) cover
both cores' needs with minimal waste (ideal is 36 tiles vs 40; the gap is
zeroed by the data-driven qpos mask). The SPMD program is identical on every
core; all per-core differences are data.

Performance structure (v2):
  - every matmul operand is fp16 (same PE rate as fp32r, half the DMA/SBUF)
  - V projection runs over the core's OWN half only (was: full sequence);
    K and V are exchanged by two pairwise AllGathers (K fires early, V later;
    P2 is restructured into scores-first/PV-second passes so the V gather
    latency hides behind all-slot score computation)
  - all weights / x tiles are multi-dim SBUF tiles filled by ONE or TWO big
    DMAs each (descriptor-generation on the issuing engine was costing
    ~630ns per 128KB tile; big transfers cut the issue count ~7x)
  - host pre-arranges every DRAM operand so each big DMA is contiguous per
    partition line (16KB runs)
  - Q/Wo/FFN matmuls interleave the two 512-token column blocks under one
    stationary weight load (halves LDWEIGHTS pressure; the pair partner's
    load hides under the 213ns FD=512 matmul)
  - P4 stages GELU results per 512-row block and stores once per block from
    the sync engine (64 -> 8 stores)
  - scalar engine runs only Identity in P1 and only Exp in P2 (activation
    table reloads cost 1.3us each)
"""

import numpy as np

D = 1024  # model dim
S = 2048  # sequence length
B = 4  # batch
M = 4096  # FFN dim
CH = 256  # q chunk (slot) size
NSLOT = 4  # q slots per core
NDT = D // 128  # 8 d-tiles
N_CORES = 8
NKT = [4, 8, 12, 16]  # k-tiles per slot (static max over the two paired cores)

_PROGRAM = None  # cached compiled program


def _build_program():
    import sys

    if "/opt/trn_rl_repo" not in sys.path:
        sys.path.insert(0, "/opt/trn_rl_repo")
    import concourse.bass as bass
    import concourse.tile as tile
    import concourse.mybir as mybir
    from concourse import bacc
    from concourse.bass import ts

    dt = mybir.dt
    AF = mybir.ActivationFunctionType
    ALU = mybir.AluOpType
    F32, F32R, F16 = dt.float32, dt.float32r, dt.float16

    nc = bacc.Bacc("TRN2", target_bir_lowering=False, debug=False, num_devices=8)

    # ---------------- DRAM I/O (all host-pre-arranged layouts) ----------------
    # weights: [128, i(8), 1024] with [p, i, c] = W.T[i*128+p, c]
    wqT = nc.dram_tensor("wqT", [128, NDT, D], F16, kind="ExternalInput").ap()
    wkT = nc.dram_tensor("wkT", [128, NDT, D], F16, kind="ExternalInput").ap()
    wvT = nc.dram_tensor("wvT", [128, NDT, D], F16, kind="ExternalInput").ap()
    woT = nc.dram_tensor("woT", [128, NDT, D], F16, kind="ExternalInput").ap()
    # FFN weight: [mb(8), 128, i(8), 512] with [mb, p, i, c] = Wf.T[i*128+p, mb*512+c]
    wfT = nc.dram_tensor("wfT", [M // 512, 128, NDT, 512], F16, kind="ExternalInput").ap()
    # x, own-half tokens in k order: [p, i, t] = x.T[i*128+p, half*1024+t]
    xaT = nc.dram_tensor("xaT", [128, NDT, S // 2], F16, kind="ExternalInput").ap()
    # x, own 4 chunks in q order
    xqT = nc.dram_tensor("xqT", [128, NDT, 4 * CH], F16, kind="ExternalInput").ap()
    bq = nc.dram_tensor("bq", [128, D // 128], F32, kind="ExternalInput").ap()
    bk = nc.dram_tensor("bk", [128, D // 128], F32, kind="ExternalInput").ap()
    bo2 = nc.dram_tensor("bo2", [128, D // 128], F32, kind="ExternalInput").ap()
    bfT = nc.dram_tensor("bfT", [128, M // 128], F32, kind="ExternalInput").ap()
    qpos = nc.dram_tensor("qpos", [1, 4 * CH], F32R, kind="ExternalInput").ap()
    iota_kt = nc.dram_tensor("iota_kt", [128, S // 128], F32, kind="ExternalInput").ap()
    # output: [mb(8), 128, mt*2+qb(8), 512] = ff^T[mb*512+mt*128+p, qb*512+q]
    ffT = nc.dram_tensor("ffT", [M // 512, 128, 8, 512], F16, kind="ExternalOutput").ap()

    with tile.TileContext(nc) as tc:
        with (
            tc.tile_pool(name="const", bufs=1) as cpool,
            tc.tile_pool(name="psum", bufs=1, space="PSUM") as pspool,
        ):
            # ---------------- constants (scalar engine issues these) --------
            ones_col_bf = cpool.tile([128, 1], F16, name="ones_col_bf", tag="ones_col_bf")
            nc.vector.memset(ones_col_bf[:], 1.0)
            ones_row_f = cpool.tile([1, 128], F32, name="ones_row_f", tag="ones_row_f")
            nc.vector.memset(ones_row_f[:], 1.0)
            ones_row = cpool.tile([1, 128], F32R, name="ones_row", tag="ones_row")
            nc.vector.tensor_copy(ones_row[:], ones_row_f[:])
            iota_sb = cpool.tile([128, S // 128], F32, name="iota", tag="iota")
            nc.scalar.dma_start(iota_sb[:], iota_kt[:])
            bq_sb = cpool.tile([128, D // 128], F32, name="bq", tag="bq")
            nc.scalar.dma_start(bq_sb[:], bq[:])
            bk_sb = cpool.tile([128, D // 128], F32, name="bk", tag="bk")
            nc.scalar.dma_start(bk_sb[:], bk[:])
            bo2_sb = cpool.tile([128, D // 128], F32, name="bo2", tag="bo2")
            nc.scalar.dma_start(bo2_sb[:], bo2[:])
            bf_sb = cpool.tile([128, M // 128], F32, name="bf", tag="bf")
            nc.scalar.dma_start(bf_sb[:], bfT[:])
            qpos_row = cpool.tile([1, 4 * CH], F32R, name="qpos_row", tag="qpos_row")
            nc.scalar.dma_start(qpos_row[:], qpos[:])
            qposB = cpool.tile([128, 4 * CH], F32, name="qposB", tag="qposB")

            # ------------- long-lived tiles: one pool spanning P1..P4 -------
            with (
                tc.tile_pool(name="main", bufs=1) as mp,
                tc.tile_pool(name="dram", bufs=1, space="DRAM") as dram,
            ):
                kT = mp.tile([128, NDT, S], F16, name="kT", tag="kT")
                vt = mp.tile([128, 16, D], F16, name="vt", tag="vt")
                wq_sb = mp.tile([128, NDT, D], F16, name="wq", tag="wq")
                wo_sb = mp.tile([128, NDT, D], F16, name="wo", tag="wo")
                xq = mp.tile([128, NDT, 4 * CH], F16, name="xq", tag="xq")
                qT = [
                    [mp.tile([128, 512], F16, name=f"qT{dt_}_{qb}", tag=f"qT{dt_}_{qb}") for qb in range(2)]
                    for dt_ in range(NDT)
                ]
                attnT = [
                    [mp.tile([128, 512], F16, name=f"at{dt_}_{qb}", tag=f"at{dt_}_{qb}") for qb in range(2)]
                    for dt_ in range(NDT)
                ]
                # DRAM bounce buffers for the pairwise K and V AllGathers
                k_in = dram.tile([128, NDT, S // 2], F16, name="k_in", tag="k_in")
                k_out = dram.tile([2, 128, NDT, S // 2], F16, name="k_out", tag="k_out")
                v_in = dram.tile([128, NDT, D], F16, name="v_in", tag="v_in")
                v_out = dram.tile([2, 128, NDT, D], F16, name="v_out", tag="v_out")

                # ---------------- P1 ----------------
                with tc.tile_pool(name="p1a", bufs=1) as p1a:
                    wk_sb = p1a.tile([128, NDT, D], F16, name="wk", tag="wk")
                    wv_sb = p1a.tile([128, NDT, D], F16, name="wv", tag="wv")
                    xa = p1a.tile([128, NDT, S // 2], F16, name="xa", tag="xa")

                    # weight streams on gpsimd (first-use order), split in two
                    # halves so the first matmuls can start early
                    for h in range(2):
                        nc.gpsimd.dma_start(wk_sb[:, ts(h, 4), :], wkT[:, ts(h, 4), :])
                    for h in range(2):
                        nc.gpsimd.dma_start(wv_sb[:, ts(h, 4), :], wvT[:, ts(h, 4), :])
                    for h in range(2):
                        nc.gpsimd.dma_start(wq_sb[:, ts(h, 4), :], wqT[:, ts(h, 4), :])
                    for h in range(2):
                        nc.gpsimd.dma_start(wo_sb[:, ts(h, 4), :], woT[:, ts(h, 4), :])
                    # x streams on sync
                    for h in range(2):
                        nc.sync.dma_start(xa[:, ts(h, 4), :], xaT[:, ts(h, 4), :])
                    nc.sync.dma_start(xq[:], xqT[:])

                    # ---- K projection over own half (kT staged at [:, :, 0:1024])
                    for tb in range(2):
                        for og in range(2):
                            ps4 = [
                                pspool.tile([128, 512], F32, name=f"at{j}", tag=f"at{j}", bufs=1)
                                for j in range(4)
                            ]
                            for i in range(NDT):
                                for j in range(4):
                                    nc.tensor.matmul(
                                        ps4[j][:],
                                        wk_sb[:, i, ts(og * 4 + j, 128)],
                                        xa[:, i, ts(tb, 512)],
                                        start=(i == 0), stop=(i == NDT - 1),
                                    )
                            for j in range(4):
                                ot = og * 4 + j
                                nc.scalar.activation(
                                    kT[:, ot, ts(tb, 512)], ps4[j][:], AF.Identity,
                                    bias=bk_sb[:, ot : ot + 1],
                                )
                    # stage own K half and fire the K gather
                    nc.sync.dma_start(k_in[:], kT[:, :, 0 : S // 2])
                    nc.gpsimd.collective_compute(
                        "AllGather",
                        mybir.AluOpType.bypass,
                        replica_groups=[[0, 1], [2, 3], [4, 5], [6, 7]],
                        ins=[k_in[:].opt()],
                        outs=[k_out[:].opt()],
                    )

                    # ---- V projection over own half (token-major, 2 banks/tt)
                    for tb in range(2):
                        for tt in range(4):
                            ps2 = [
                                pspool.tile([128, 512], F32, name=f"mm{ob}", tag="mm", bufs=3)
                                for ob in range(2)
                            ]
                            for i in range(NDT):
                                for ob in range(2):
                                    nc.tensor.matmul(
                                        ps2[ob][:],
                                        xa[:, i, tb * 512 + tt * 128 : tb * 512 + (tt + 1) * 128],
                                        wv_sb[:, i, ts(ob, 512)],
                                        start=(i == 0), stop=(i == NDT - 1),
                                    )
                            for ob in range(2):
                                nc.scalar.activation(
                                    vt[:, tb * 4 + tt, ts(ob, 512)], ps2[ob][:], AF.Identity
                                )
                    # stage own V half (scalar: right after its own drains) and
                    # gather. vt[:, 0:8] doubles as the projection scratch; the
                    # readback below overwrites all 16 slots in token order.
                    nc.scalar.dma_start(v_in[:], vt[:, 0:NDT, :])
                    nc.gpsimd.collective_compute(
                        "AllGather",
                        mybir.AluOpType.bypass,
                        replica_groups=[[0, 1], [2, 3], [4, 5], [6, 7]],
                        ins=[v_in[:].opt()],
                        outs=[v_out[:].opt()],
                    )

                    # qpos broadcast (fills the gather window)
                    for i in range(4 * CH // 512):
                        bc_ps = pspool.tile([128, 512], F32, name="small", tag="small", bufs=1)
                        nc.tensor.matmul(
                            bc_ps[:], ones_row[:], qpos_row[:, ts(i, 512)],
                            start=True, stop=True,
                        )
                        nc.scalar.activation(qposB[:, ts(i, 512)], bc_ps[:], AF.Identity)

                    # ---- Q projection: qb pair interleaved under one weight
                    for ot in range(NDT):
                        psq = [
                            pspool.tile([128, 512], F32, name=f"mmq{qb}", tag="mm", bufs=3)
                            for qb in range(2)
                        ]
                        for i in range(NDT):
                            for qb in range(2):
                                nc.tensor.matmul(
                                    psq[qb][:],
                                    wq_sb[:, i, ts(ot, 128)],
                                    xq[:, i, ts(qb, 512)],
                                    start=(i == 0), stop=(i == NDT - 1),
                                )
                        for qb in range(2):
                            nc.scalar.activation(
                                qT[ot][qb][:], psq[qb][:], AF.Identity,
                                bias=bq_sb[:, ot : ot + 1],
                            )

                # gathered K^T / V readback in true token order, ordered by
                # first use in P2 (sync engine; waits ride on the collectives)
                for r in range(2):
                    nc.sync.dma_start(
                        kT[:, :, r * (S // 2) : (r + 1) * (S // 2)], k_out[r]
                    )
                nc.sync.dma_start(vt[:, 0:4, :], v_out[0, :, 0:4, :])
                nc.sync.dma_start(vt[:, 4:8, :], v_out[0, :, 4:8, :])
                nc.sync.dma_start(vt[:, 8:16, :], v_out[1])

                # ---------------- P2: attention ----------------
                # pass A: scores + exp + mask + denom + recip for ALL slots
                # (keeps probs resident; PV waits for the V gather in pass B)
                with tc.tile_pool(name="p2", bufs=1) as p2:
                    pt = [
                        [
                            p2.tile([128, CH], F16, name=f"pt{sl}_{k}", tag=f"pt{sl}_{k}", bufs=1)
                            for k in range(NKT[sl])
                        ]
                        for sl in range(NSLOT)
                    ]
                    recipB = [
                        p2.tile([128, CH], F32, name=f"recipB{sl}", tag=f"recipB{sl}", bufs=1)
                        for sl in range(NSLOT)
                    ]
                    for sl in range(NSLOT):
                        nkt = NKT[sl]
                        dn_ps = pspool.tile([1, CH], F32, name="small", tag="small", bufs=1)
                        for k in range(nkt):
                            ps = pspool.tile([128, CH], F32, name="mm", tag="mm", bufs=3)
                            for i in range(NDT):
                                nc.tensor.matmul(
                                    ps[:],
                                    kT[:, i, ts(k, 128)],
                                    qT[i][sl // 2][:, ts(sl % 2, CH)],
                                    start=(i == 0), stop=(i == NDT - 1),
                                )
                            masked = k >= nkt - 4
                            if masked:
                                praw = p2.tile([128, CH], F16, name="praw", tag="praw", bufs=2)
                                nc.scalar.activation(
                                    praw[:], ps[:], AF.Exp, scale=1.0 / 32.0
                                )
                                msk = p2.tile([128, CH], F16, name="msk", tag="msk", bufs=2)
                                nc.vector.tensor_scalar(
                                    out=msk[:],
                                    in0=qposB[:, ts(sl, CH)],
                                    scalar1=iota_sb[:, k : k + 1],
                                    scalar2=None,
                                    op0=ALU.is_ge,
                                )
                                nc.vector.tensor_tensor(
                                    out=pt[sl][k][:], in0=praw[:], in1=msk[:],
                                    op=ALU.mult,
                                )
                            else:
                                nc.scalar.activation(
                                    pt[sl][k][:], ps[:], AF.Exp, scale=1.0 / 32.0
                                )
                            nc.tensor.matmul(
                                dn_ps[:], ones_col_bf[:], pt[sl][k][:],
                                start=(k == 0), stop=(k == nkt - 1),
                            )
                        dn_r = p2.tile([1, CH], F32R, name="dn_r", tag="dn_r", bufs=2)
                        nc.vector.tensor_copy(dn_r[:], dn_ps[:])
                        rb_ps = pspool.tile([128, CH], F32, name="small", tag="small", bufs=1)
                        nc.tensor.matmul(
                            rb_ps[:], ones_row[:], dn_r[:], start=True, stop=True
                        )
                        nc.vector.reciprocal(recipB[sl][:], rb_ps[:])

                    # pass B: attn^T = (P @ V)^T scaled by 1/denom
                    for sl in range(NSLOT):
                        nkt = NKT[sl]
                        for half in range(2):
                            for d4 in range(4):
                                d_ = half * 4 + d4
                                aps = pspool.tile([128, CH], F32, name=f"at{d4}", tag=f"at{d4}", bufs=1)
                                for k in range(nkt):
                                    nc.tensor.matmul(
                                        aps[:],
                                        vt[:, k, ts(d_, 128)],
                                        pt[sl][k][:],
                                        start=(k == 0), stop=(k == nkt - 1),
                                    )
                                nc.vector.tensor_tensor(
                                    out=attnT[d_][sl // 2][:, ts(sl % 2, CH)],
                                    in0=aps[:], in1=recipB[sl][:],
                                    op=ALU.mult,
                                )

                # ---------------- P3 + P4 ----------------
                with tc.tile_pool(name="p34", bufs=1) as p34:
                    outT = [
                        [p34.tile([128, 512], F16, name=f"oT{dt_}_{qb}", tag=f"oT{dt_}_{qb}") for qb in range(2)]
                        for dt_ in range(NDT)
                    ]
                    # P3: qb pair interleaved under one Wo weight block
                    for ot in range(NDT):
                        pso = [
                            pspool.tile([128, 512], F32, name=f"mmo{qb}", tag="mm", bufs=3)
                            for qb in range(2)
                        ]
                        for i in range(NDT):
                            for qb in range(2):
                                nc.tensor.matmul(
                                    pso[qb][:],
                                    wo_sb[:, i, ts(ot, 128)],
                                    attnT[i][qb][:],
                                    start=(i == 0), stop=(i == NDT - 1),
                                )
                        for qb in range(2):
                            nc.scalar.activation(
                                outT[ot][qb][:], pso[qb][:], AF.Identity,
                                bias=bo2_sb[:, ot : ot + 1],
                            )

                    # P4: FFN + GELU; one weight DMA and one store per mb block
                    for mb in range(M // 512):
                        wfb = p34.tile([128, NDT, 512], F16, name="wfb", tag="wfb", bufs=2)
                        nc.gpsimd.dma_start(wfb[:], wfT[mb])
                        st = p34.tile([128, 8, 512], F16, name="ffstage", tag="ffstage", bufs=2)
                        for mt in range(4):
                            m = mb * 4 + mt
                            psf = [
                                pspool.tile([128, 512], F32, name=f"mmf{qb}", tag="mm", bufs=3)
                                for qb in range(2)
                            ]
                            for i in range(NDT):
                                for qb in range(2):
                                    nc.tensor.matmul(
                                        psf[qb][:],
                                        wfb[:, i, ts(mt, 128)],
                                        outT[i][qb][:],
                                        start=(i == 0), stop=(i == NDT - 1),
                                    )
                            for qb in range(2):
                                nc.scalar.activation(
                                    st[:, mt * 2 + qb, :], psf[qb][:], AF.Gelu,
                                    bias=bf_sb[:, m : m + 1],
                                )
                        nc.sync.dma_start(ffT[mb], st[:])

    nc.compile()
    return nc


def _get_program():
    global _PROGRAM
    if _PROGRAM is None:
        _PROGRAM = _build_program()
    return _PROGRAM


def _owned_chunks(core):
    """The four 256-token chunk indices this core owns, in slot order."""
    if core % 2 == 0:
        return (0, 3, 4, 7)
    return (1, 2, 5, 6)


def _blocked(a):
    """[1024, W] -> [128, 8, W] with [p, i, c] = a[i*128+p, c]."""
    W = a.shape[1]
    return np.ascontiguousarray(a.reshape(8, 128, W).transpose(1, 0, 2))


def _make_in_maps(x, Wq, bq, Wk, bk, Wv, bv, Wo, bo, Wf, bf):
    f32, f16 = np.float32, np.float16
    wqT = _blocked(np.asarray(Wq.T, dtype=f16))
    wkT = _blocked(np.asarray(Wk.T, dtype=f16))
    wvT = _blocked(np.asarray(Wv.T, dtype=f16))
    woT = _blocked(np.asarray(Wo.T, dtype=f16))
    # wfT[mb, p, i, c] = Wf.T[i*128+p, mb*512+c]
    wfT = np.ascontiguousarray(
        np.asarray(Wf.T, dtype=f16).reshape(8, 128, 8, 512).transpose(2, 1, 0, 3)
    )
    bo2 = (Wo.astype(np.float64) @ bv.astype(np.float64) + bo.astype(np.float64))
    bo2 = np.ascontiguousarray(bo2.astype(f32).reshape(D // 128, 128).T)
    bfT = np.ascontiguousarray(bf.reshape(M // 128, 128).T, dtype=f32)
    iota = (
        np.arange(128, dtype=f32)[:, None]
        + 128.0 * np.arange(S // 128, dtype=f32)[None, :]
    )
    shared = {
        "wqT": wqT, "wkT": wkT, "wvT": wvT, "woT": woT, "wfT": wfT,
        "bq": np.ascontiguousarray(bq.reshape(D // 128, 128).T, dtype=f32),
        "bk": np.ascontiguousarray(bk.reshape(D // 128, 128).T, dtype=f32),
        "bo2": bo2,
        "bfT": bfT,
        "iota_kt": np.ascontiguousarray(iota),
    }
    in_maps = []
    for core in range(N_CORES):
        b = core // 2
        chunks = _owned_chunks(core)
        xTb = np.asarray(x[b].T, dtype=f16)  # [D, S]
        half = core % 2  # rank within the pair: rank0 owns tokens 0:S/2
        xaT = _blocked(xTb[:, half * (S // 2) : (half + 1) * (S // 2)])
        xqT = _blocked(
            np.concatenate([xTb[:, c * CH : (c + 1) * CH] for c in chunks], axis=1)
        )
        qp = np.concatenate(
            [np.arange(c * CH, (c + 1) * CH) for c in chunks]
        ).astype(f32)[None, :]
        in_maps.append(
            {**shared, "xaT": xaT, "xqT": xqT,
             "qpos": np.ascontiguousarray(qp)}
        )
    return in_maps


def _run(inputs, trace=False, trace_cores=None, tmpdir=None):
    import sys

    if "/opt/trn_rl_repo" not in sys.path:
        sys.path.insert(0, "/opt/trn_rl_repo")
    from concourse.bass_utils import run_bass_kernel_spmd

    nc = _get_program()
    in_maps = _make_in_maps(**inputs)
    res = run_bass_kernel_spmd(
        nc, in_maps, list(range(N_CORES)), trace=trace, trace_cores=trace_cores,
        tmpdir=tmpdir,
    )
    out = np.empty((B, S, M), dtype=np.float32)
    for core in range(N_CORES):
        b = core // 2
        chunks = _owned_chunks(core)
        # ffT[mb, p, mt*2+qb, q] = ff^T[mb*512+mt*128+p, qb*512+q]
        raw = res.results[core]["ffT"].reshape(8, 128, 4, 2, 512)
        ffT = np.ascontiguousarray(
            raw.transpose(0, 2, 1, 3, 4)
        ).reshape(M, 4 * CH)
        for sl, c in enumerate(chunks):
            qb, qo = divmod(sl, 2)
            out[b, c * CH : (c + 1) * CH] = (
                ffT[:, qb * 512 + qo * CH : qb * 512 + (qo + 1) * CH].T.astype(np.float32)
            )
    return out, res


def kernel(**inputs):
    out, _ = _run(inputs)
    return out


# revision 13
# speedup vs baseline: 1.1472x; 1.1102x over previous
"""Decoder block (single-head causal attention + GELU FFN) on 8 TRN2 NeuronCores.

Sharding: data parallel over batch (2 cores per batch), with the K AND V
projections token-split across the pair (each core projects its own half of
the sequence, then a pairwise AllGather shares both). Core c handles batch
b = c//2 and 1024 query tokens of that batch, chosen as four 256-token chunks
that balance the causal-attention workload:
  even cores (half 0): chunks 0, 3, 4, 7
  odd  cores (half 1): chunks 1, 2, 5, 6
The slot pairing makes the static k-tile counts per slot (4, 8, 12, 16) cover
both cores' needs with minimal waste (ideal is 36 tiles vs 40; the gap is
zeroed by the data-driven qpos mask). The SPMD program is identical on every
core; all per-core differences are data.

Performance structure (v2):
  - every matmul operand is fp16 (same PE rate as fp32r, half the DMA/SBUF)
  - V projection runs over the core's OWN half only (was: full sequence);
    K and V are exchanged by two pairwise AllGathers (K fires early, V later;
    P2 is restructured into scores-first/PV-second passes so the V gather
    latency hides behind all-slot score computation)
  - all weights / x tiles are multi-dim SBUF tiles filled by ONE or TWO big
    DMAs each (descriptor-generation on the issuing engine was costing
    ~630ns per 128KB tile; big transfers cut the issue count ~7x)
  - host pre-arranges every DRAM operand so each big DMA is contiguous per
    partition line (16KB runs)
  - Q/Wo/FFN matmuls interleave the two 512-token column blocks under one
    stationary weight load (halves LDWEIGHTS pressure; the pair partner's
    load hides under the 213ns FD=512 matmul)
  - P4 stages GELU results per 512-row block and stores once per block from
    the sync engine (64 -> 8 stores)
  - scalar engine runs only Identity in P1 and only Exp in P2 (activation
    table reloads cost 1.3us each)
"""

import numpy as np

D = 1024  # model dim
S = 2048  # sequence length
B = 4  # batch
M = 4096  # FFN dim
CH = 256  # q chunk (slot) size
NSLOT = 4  # q slots per core
NDT = D // 128  # 8 d-tiles
N_CORES = 8
NKT = [4, 8, 12, 16]  # k-tiles per slot (static max over the two paired cores)

_PROGRAM = None  # cached compiled program


def _build_program():
    import sys

    if "/opt/trn_rl_repo" not in sys.path:
        sys.path.insert(0, "/opt/trn_rl_repo")
    import concourse.bass as bass
    import concourse.tile as tile
    import concourse.mybir as mybir
    from concourse import bacc
    from concourse.bass import ts

    dt = mybir.dt
    AF = mybir.ActivationFunctionType
    ALU = mybir.AluOpType
    F32, F32R, F16 = dt.float32, dt.float32r, dt.float16

    nc = bacc.Bacc("TRN2", target_bir_lowering=False, debug=False, num_devices=8)

    # ---------------- DRAM I/O (all host-pre-arranged layouts) ----------------
    # weights: [128, i(8), 1024] with [p, i, c] = W.T[i*128+p, c]
    wqT = nc.dram_tensor("wqT", [128, NDT, D], F16, kind="ExternalInput").ap()
    wkT = nc.dram_tensor("wkT", [128, NDT, D], F16, kind="ExternalInput").ap()
    wvT = nc.dram_tensor("wvT", [128, NDT, D], F16, kind="ExternalInput").ap()
    woT = nc.dram_tensor("woT", [128, NDT, D], F16, kind="ExternalInput").ap()
    # FFN weight: [mb(8), 128, i(8), 512] with [mb, p, i, c] = Wf.T[i*128+p, mb*512+c]
    wfT = nc.dram_tensor("wfT", [M // 512, 128, NDT, 512], F16, kind="ExternalInput").ap()
    # x, own-half tokens in k order: [p, i, t] = x.T[i*128+p, half*1024+t]
    xaT = nc.dram_tensor("xaT", [128, NDT, S // 2], F16, kind="ExternalInput").ap()
    # x, own 4 chunks in q order
    xqT = nc.dram_tensor("xqT", [128, NDT, 4 * CH], F16, kind="ExternalInput").ap()
    bq = nc.dram_tensor("bq", [128, D // 128], F32, kind="ExternalInput").ap()
    bk = nc.dram_tensor("bk", [128, D // 128], F32, kind="ExternalInput").ap()
    bo2 = nc.dram_tensor("bo2", [128, D // 128], F32, kind="ExternalInput").ap()
    bfT = nc.dram_tensor("bfT", [128, M // 128], F32, kind="ExternalInput").ap()
    qpos = nc.dram_tensor("qpos", [1, 4 * CH], F32R, kind="ExternalInput").ap()
    iota_kt = nc.dram_tensor("iota_kt", [128, S // 128], F32, kind="ExternalInput").ap()
    # output: [mb(8), 128, mt*2+qb(8), 512] = ff^T[mb*512+mt*128+p, qb*512+q]
    ffT = nc.dram_tensor("ffT", [M // 512, 128, 8, 512], F16, kind="ExternalOutput").ap()

    with tile.TileContext(nc) as tc:
        with (
            tc.tile_pool(name="const", bufs=1) as cpool,
            tc.tile_pool(name="psum", bufs=1, space="PSUM") as pspool,
        ):
            # ---------------- constants (scalar engine issues these) --------
            ones_col_bf = cpool.tile([128, 1], F16, name="ones_col_bf", tag="ones_col_bf")
            nc.vector.memset(ones_col_bf[:], 1.0)
            ones_row_f = cpool.tile([1, 128], F32, name="ones_row_f", tag="ones_row_f")
            nc.vector.memset(ones_row_f[:], 1.0)
            ones_row = cpool.tile([1, 128], F32R, name="ones_row", tag="ones_row")
            nc.vector.tensor_copy(ones_row[:], ones_row_f[:])
            iota_sb = cpool.tile([128, S // 128], F32, name="iota", tag="iota")
            nc.scalar.dma_start(iota_sb[:], iota_kt[:])
            bq_sb = cpool.tile([128, D // 128], F32, name="bq", tag="bq")
            nc.scalar.dma_start(bq_sb[:], bq[:])
            bk_sb = cpool.tile([128, D // 128], F32, name="bk", tag="bk")
            nc.scalar.dma_start(bk_sb[:], bk[:])
            bo2_sb = cpool.tile([128, D // 128], F32, name="bo2", tag="bo2")
            nc.scalar.dma_start(bo2_sb[:], bo2[:])
            bf_sb = cpool.tile([128, M // 128], F32, name="bf", tag="bf")
            nc.scalar.dma_start(bf_sb[:], bfT[:])
            qpos_row = cpool.tile([1, 4 * CH], F32R, name="qpos_row", tag="qpos_row")
            nc.scalar.dma_start(qpos_row[:], qpos[:])
            qposB = cpool.tile([128, 4 * CH], F32, name="qposB", tag="qposB")

            # ------------- long-lived tiles: one pool spanning P1..P4 -------
            with (
                tc.tile_pool(name="main", bufs=1) as mp,
                tc.tile_pool(name="dram", bufs=1, space="DRAM") as dram,
            ):
                kT = mp.tile([128, NDT, S], F16, name="kT", tag="kT")
                vt = mp.tile([128, 16, D], F16, name="vt", tag="vt")
                wq_sb = mp.tile([128, NDT, D], F16, name="wq", tag="wq")
                wo_sb = mp.tile([128, NDT, D], F16, name="wo", tag="wo")
                xq = mp.tile([128, NDT, 4 * CH], F16, name="xq", tag="xq")
                qT = [
                    [mp.tile([128, 512], F16, name=f"qT{dt_}_{qb}", tag=f"qT{dt_}_{qb}") for qb in range(2)]
                    for dt_ in range(NDT)
                ]
                attnT = [
                    [mp.tile([128, 512], F16, name=f"at{dt_}_{qb}", tag=f"at{dt_}_{qb}") for qb in range(2)]
                    for dt_ in range(NDT)
                ]
                # DRAM bounce buffers for the pairwise K and V AllGathers.
                # Each projection is gathered in two 1MB halves so the
                # collectives fire earlier and finish well before P2 needs
                # the peer's tokens.
                ka_in = dram.tile([128, NDT, 512], F16, name="ka_in", tag="ka_in")
                ka_out = dram.tile([2, 128, NDT, 512], F16, name="ka_out", tag="ka_out")
                kb_in = dram.tile([128, NDT, 512], F16, name="kb_in", tag="kb_in")
                kb_out = dram.tile([2, 128, NDT, 512], F16, name="kb_out", tag="kb_out")
                va_in = dram.tile([128, 4, D], F16, name="va_in", tag="va_in")
                va_out = dram.tile([2, 128, 4, D], F16, name="va_out", tag="va_out")
                vb_in = dram.tile([128, 4, D], F16, name="vb_in", tag="vb_in")
                vb_out = dram.tile([2, 128, 4, D], F16, name="vb_out", tag="vb_out")

                def pair_gather(in_t, out_t):
                    nc.gpsimd.collective_compute(
                        "AllGather",
                        mybir.AluOpType.bypass,
                        replica_groups=[[0, 1], [2, 3], [4, 5], [6, 7]],
                        ins=[in_t[:].opt()],
                        outs=[out_t[:].opt()],
                    )

                # ---------------- P1 ----------------
                with tc.tile_pool(name="p1a", bufs=1) as p1a:
                    wk_sb = p1a.tile([128, NDT, D], F16, name="wk", tag="wk")
                    wv_sb = p1a.tile([128, NDT, D], F16, name="wv", tag="wv")
                    xa = p1a.tile([128, NDT, S // 2], F16, name="xa", tag="xa")

                    # DMA issue is tiered: all in-flight DMAs share wire
                    # bandwidth, so only the critical-path wk/xa stream starts
                    # immediately (per-i granularity: compute starts on the
                    # first 256KB). Later streams are gated on compute
                    # sentinels and issued from the otherwise-idle vector
                    # engine so they cannot steal bandwidth early.
                    for i in range(NDT):
                        nc.gpsimd.dma_start(wk_sb[:, i, :], wkT[:, i, :])
                        nc.sync.dma_start(xa[:, i, :], xaT[:, i, :])

                    # ---- K projection over own half (kT staged at [:, :, 0:1024])
                    for tb in range(2):
                        for og in range(2):
                            ps4 = [
                                pspool.tile([128, 512], F32, name=f"at{j}", tag=f"at{j}", bufs=1)
                                for j in range(4)
                            ]
                            for i in range(NDT):
                                for j in range(4):
                                    nc.tensor.matmul(
                                        ps4[j][:],
                                        wk_sb[:, i, ts(og * 4 + j, 128)],
                                        xa[:, i, ts(tb, 512)],
                                        start=(i == 0), stop=(i == NDT - 1),
                                    )
                            for j in range(4):
                                ot = og * 4 + j
                                nc.scalar.activation(
                                    kT[:, ot, ts(tb, 512)], ps4[j][:], AF.Identity,
                                    bias=bk_sb[:, ot : ot + 1],
                                )
                            if tb == 0 and og == 0:
                                # tier-1 issue: scalar reaches here only after
                                # og0's drains, so wv/xq cannot steal wire
                                # bandwidth from the critical wk/xa stream
                                for h in range(2):
                                    nc.scalar.dma_start(
                                        wv_sb[:, ts(h, 4), :], wvT[:, ts(h, 4), :]
                                    )
                                nc.scalar.dma_start(xq[:], xqT[:])
                        # stage this token block and fire its K gather
                        kin = ka_in if tb == 0 else kb_in
                        nc.sync.dma_start(kin[:], kT[:, :, ts(tb, 512)])
                        pair_gather(kin, ka_out if tb == 0 else kb_out)

                    # ---- V projection over own half (token-major, 2 banks/tt)
                    for tb in range(2):
                        for tt in range(4):
                            ps2 = [
                                pspool.tile([128, 512], F32, name=f"mm{ob}", tag="mm", bufs=3)
                                for ob in range(2)
                            ]
                            for i in range(NDT):
                                for ob in range(2):
                                    nc.tensor.matmul(
                                        ps2[ob][:],
                                        xa[:, i, tb * 512 + tt * 128 : tb * 512 + (tt + 1) * 128],
                                        wv_sb[:, i, ts(ob, 512)],
                                        start=(i == 0), stop=(i == NDT - 1),
                                    )
                            for ob in range(2):
                                nc.scalar.activation(
                                    vt[:, tb * 4 + tt, ts(ob, 512)], ps2[ob][:], AF.Identity
                                )
                            if tb == 0 and tt == 0:
                                # tier-2 issue (see tier-1 note)
                                for h in range(2):
                                    nc.scalar.dma_start(
                                        wq_sb[:, ts(h, 4), :], wqT[:, ts(h, 4), :]
                                    )
                                for h in range(2):
                                    nc.scalar.dma_start(
                                        wo_sb[:, ts(h, 4), :], woT[:, ts(h, 4), :]
                                    )
                        # stage this V token block and fire its gather.
                        # vt[:, 0:8] doubles as the projection scratch; the
                        # readback overwrites all 16 slots in token order.
                        vin = va_in if tb == 0 else vb_in
                        nc.scalar.dma_start(vin[:], vt[:, ts(tb, 4), :])
                        pair_gather(vin, va_out if tb == 0 else vb_out)

                    # qpos broadcast (fills the gather window)
                    for i in range(4 * CH // 512):
                        bc_ps = pspool.tile([128, 512], F32, name="small", tag="small", bufs=1)
                        nc.tensor.matmul(
                            bc_ps[:], ones_row[:], qpos_row[:, ts(i, 512)],
                            start=True, stop=True,
                        )
                        nc.scalar.activation(qposB[:, ts(i, 512)], bc_ps[:], AF.Identity)

                    # ---- Q projection: qb pair interleaved under one weight
                    for ot in range(NDT):
                        psq = [
                            pspool.tile([128, 512], F32, name=f"mmq{qb}", tag="mm", bufs=3)
                            for qb in range(2)
                        ]
                        for i in range(NDT):
                            for qb in range(2):
                                nc.tensor.matmul(
                                    psq[qb][:],
                                    wq_sb[:, i, ts(ot, 128)],
                                    xq[:, i, ts(qb, 512)],
                                    start=(i == 0), stop=(i == NDT - 1),
                                )
                        for qb in range(2):
                            nc.scalar.activation(
                                qT[ot][qb][:], psq[qb][:], AF.Identity,
                                bias=bq_sb[:, ot : ot + 1],
                            )

                # gathered K^T / V readback in true token order, ordered by
                # first use in P2 (sync engine; waits ride on the collectives)
                for r in range(2):
                    nc.sync.dma_start(
                        kT[:, :, r * 1024 + 0 : r * 1024 + 512], ka_out[r]
                    )
                    nc.sync.dma_start(
                        kT[:, :, r * 1024 + 512 : r * 1024 + 1024], kb_out[r]
                    )
                for r in range(2):
                    nc.sync.dma_start(vt[:, r * 8 + 0 : r * 8 + 4, :], va_out[r])
                    nc.sync.dma_start(vt[:, r * 8 + 4 : r * 8 + 8, :], vb_out[r])

                # ---------------- P2: attention ----------------
                # pass A: scores + exp + mask + denom + recip for ALL slots
                # (keeps probs resident; PV waits for the V gather in pass B)
                with tc.tile_pool(name="p2", bufs=1) as p2:
                    pt = [
                        [
                            p2.tile([128, CH], F16, name=f"pt{sl}_{k}", tag=f"pt{sl}_{k}", bufs=1)
                            for k in range(NKT[sl])
                        ]
                        for sl in range(NSLOT)
                    ]
                    recipB = [
                        p2.tile([128, CH], F32, name=f"recipB{sl}", tag=f"recipB{sl}", bufs=1)
                        for sl in range(NSLOT)
                    ]
                    for sl in range(NSLOT):
                        nkt = NKT[sl]
                        # all score matmuls first (exp/mask ride on scalar and
                        # vector behind them), THEN the denominator chain: the
                        # in-order PE never stalls on a scalar exp mid-stream
                        for k in range(nkt):
                            ps = pspool.tile([128, CH], F32, name="mm", tag="mm", bufs=3)
                            for i in range(NDT):
                                nc.tensor.matmul(
                                    ps[:],
                                    kT[:, i, ts(k, 128)],
                                    qT[i][sl // 2][:, ts(sl % 2, CH)],
                                    start=(i == 0), stop=(i == NDT - 1),
                                )
                            masked = k >= nkt - 4
                            if masked:
                                praw = p2.tile([128, CH], F16, name="praw", tag="praw", bufs=2)
                                nc.scalar.activation(
                                    praw[:], ps[:], AF.Exp, scale=1.0 / 32.0
                                )
                                msk = p2.tile([128, CH], F16, name="msk", tag="msk", bufs=2)
                                nc.vector.tensor_scalar(
                                    out=msk[:],
                                    in0=qposB[:, ts(sl, CH)],
                                    scalar1=iota_sb[:, k : k + 1],
                                    scalar2=None,
                                    op0=ALU.is_ge,
                                )
                                nc.vector.tensor_tensor(
                                    out=pt[sl][k][:], in0=praw[:], in1=msk[:],
                                    op=ALU.mult,
                                )
                            else:
                                nc.scalar.activation(
                                    pt[sl][k][:], ps[:], AF.Exp, scale=1.0 / 32.0
                                )
                        dn_ps = pspool.tile([1, CH], F32, name="small", tag="small", bufs=1)
                        for k in range(nkt):
                            nc.tensor.matmul(
                                dn_ps[:], ones_col_bf[:], pt[sl][k][:],
                                start=(k == 0), stop=(k == nkt - 1),
                            )
                        dn_r = p2.tile([1, CH], F32R, name="dn_r", tag="dn_r", bufs=2)
                        nc.vector.tensor_copy(dn_r[:], dn_ps[:])
                        rb_ps = pspool.tile([128, CH], F32, name="small", tag="small", bufs=1)
                        nc.tensor.matmul(
                            rb_ps[:], ones_row[:], dn_r[:], start=True, stop=True
                        )
                        nc.vector.reciprocal(recipB[sl][:], rb_ps[:])

                    # pass B: attn^T = (P @ V)^T scaled by 1/denom
                    for sl in range(NSLOT):
                        nkt = NKT[sl]
                        for half in range(2):
                            for d4 in range(4):
                                d_ = half * 4 + d4
                                aps = pspool.tile([128, CH], F32, name=f"at{d4}", tag=f"at{d4}", bufs=1)
                                for k in range(nkt):
                                    nc.tensor.matmul(
                                        aps[:],
                                        vt[:, k, ts(d_, 128)],
                                        pt[sl][k][:],
                                        start=(k == 0), stop=(k == nkt - 1),
                                    )
                                nc.vector.tensor_tensor(
                                    out=attnT[d_][sl // 2][:, ts(sl % 2, CH)],
                                    in0=aps[:], in1=recipB[sl][:],
                                    op=ALU.mult,
                                )

                # ---------------- P3 + P4 ----------------
                with tc.tile_pool(name="p34", bufs=1) as p34:
                    outT = [
                        [p34.tile([128, 512], F16, name=f"oT{dt_}_{qb}", tag=f"oT{dt_}_{qb}") for qb in range(2)]
                        for dt_ in range(NDT)
                    ]
                    # P3: qb pair interleaved under one Wo weight block
                    for ot in range(NDT):
                        pso = [
                            pspool.tile([128, 512], F32, name=f"mmo{qb}", tag="mm", bufs=3)
                            for qb in range(2)
                        ]
                        for i in range(NDT):
                            for qb in range(2):
                                nc.tensor.matmul(
                                    pso[qb][:],
                                    wo_sb[:, i, ts(ot, 128)],
                                    attnT[i][qb][:],
                                    start=(i == 0), stop=(i == NDT - 1),
                                )
                        for qb in range(2):
                            nc.scalar.activation(
                                outT[ot][qb][:], pso[qb][:], AF.Identity,
                                bias=bo2_sb[:, ot : ot + 1],
                            )

                    # P4: FFN + GELU; one weight DMA and one store per mb block
                    for mb in range(M // 512):
                        wfb = p34.tile([128, NDT, 512], F16, name="wfb", tag="wfb", bufs=2)
                        nc.gpsimd.dma_start(wfb[:], wfT[mb])
                        for mt in range(4):
                            m = mb * 4 + mt
                            psf = [
                                pspool.tile([128, 512], F32, name=f"mmf{qb}", tag="mm", bufs=3)
                                for qb in range(2)
                            ]
                            for i in range(NDT):
                                for qb in range(2):
                                    nc.tensor.matmul(
                                        psf[qb][:],
                                        wfb[:, i, ts(mt, 128)],
                                        outT[i][qb][:],
                                        start=(i == 0), stop=(i == NDT - 1),
                                    )
                            st = p34.tile([128, 2, 512], F16, name="ffstage", tag="ffstage", bufs=4)
                            for qb in range(2):
                                nc.scalar.activation(
                                    st[:, qb, :], psf[qb][:], AF.Gelu,
                                    bias=bf_sb[:, m : m + 1],
                                )
                            # fine-grained stores: the last tile's store tail
                            # is 256KB, not 1MB
                            nc.sync.dma_start(ffT[mb, :, ts(mt, 2), :], st[:])

    nc.compile()
    return nc


def _get_program():
    global _PROGRAM
    if _PROGRAM is None:
        _PROGRAM = _build_program()
    return _PROGRAM


def _owned_chunks(core):
    """The four 256-token chunk indices this core owns, in slot order."""
    if core % 2 == 0:
        return (0, 3, 4, 7)
    return (1, 2, 5, 6)


def _blocked(a):
    """[1024, W] -> [128, 8, W] with [p, i, c] = a[i*128+p, c]."""
    W = a.shape[1]
    return np.ascontiguousarray(a.reshape(8, 128, W).transpose(1, 0, 2))


def _make_in_maps(x, Wq, bq, Wk, bk, Wv, bv, Wo, bo, Wf, bf):
    f32, f16 = np.float32, np.float16
    wqT = _blocked(np.asarray(Wq.T, dtype=f16))
    wkT = _blocked(np.asarray(Wk.T, dtype=f16))
    wvT = _blocked(np.asarray(Wv.T, dtype=f16))
    woT = _blocked(np.asarray(Wo.T, dtype=f16))
    # wfT[mb, p, i, c] = Wf.T[i*128+p, mb*512+c]
    wfT = np.ascontiguousarray(
        np.asarray(Wf.T, dtype=f16).reshape(8, 128, 8, 512).transpose(2, 1, 0, 3)
    )
    bo2 = (Wo.astype(np.float64) @ bv.astype(np.float64) + bo.astype(np.float64))
    bo2 = np.ascontiguousarray(bo2.astype(f32).reshape(D // 128, 128).T)
    bfT = np.ascontiguousarray(bf.reshape(M // 128, 128).T, dtype=f32)
    iota = (
        np.arange(128, dtype=f32)[:, None]
        + 128.0 * np.arange(S // 128, dtype=f32)[None, :]
    )
    shared = {
        "wqT": wqT, "wkT": wkT, "wvT": wvT, "woT": woT, "wfT": wfT,
        "bq": np.ascontiguousarray(bq.reshape(D // 128, 128).T, dtype=f32),
        "bk": np.ascontiguousarray(bk.reshape(D // 128, 128).T, dtype=f32),
        "bo2": bo2,
        "bfT": bfT,
        "iota_kt": np.ascontiguousarray(iota),
    }
    in_maps = []
    for core in range(N_CORES):
        b = core // 2
        chunks = _owned_chunks(core)
        xTb = np.asarray(x[b].T, dtype=f16)  # [D, S]
        half = core % 2  # rank within the pair: rank0 owns tokens 0:S/2
        xaT = _blocked(xTb[:, half * (S // 2) : (half + 1) * (S // 2)])
        xqT = _blocked(
            np.concatenate([xTb[:, c * CH : (c + 1) * CH] for c in chunks], axis=1)
        )
        qp = np.concatenate(
            [np.arange(c * CH, (c + 1) * CH) for c in chunks]
        ).astype(f32)[None, :]
        in_maps.append(
            {**shared, "xaT": xaT, "xqT": xqT,
             "qpos": np.ascontiguousarray(qp)}
        )
    return in_maps


def _run(inputs, trace=False, trace_cores=None, tmpdir=None):
    import sys

    if "/opt/trn_rl_repo" not in sys.path:
        sys.path.insert(0, "/opt/trn_rl_repo")
    from concourse.bass_utils import run_bass_kernel_spmd

    nc = _get_program()
    in_maps = _make_in_maps(**inputs)
    res = run_bass_kernel_spmd(
        nc, in_maps, list(range(N_CORES)), trace=trace, trace_cores=trace_cores,
        tmpdir=tmpdir,
    )
    out = np.empty((B, S, M), dtype=np.float32)
    for core in range(N_CORES):
        b = core // 2
        chunks = _owned_chunks(core)
        # ffT[mb, p, mt*2+qb, q] = ff^T[mb*512+mt*128+p, qb*512+q]
        raw = res.results[core]["ffT"].reshape(8, 128, 4, 2, 512)
        ffT = np.ascontiguousarray(
            raw.transpose(0, 2, 1, 3, 4)
        ).reshape(M, 4 * CH)
        for sl, c in enumerate(chunks):
            qb, qo = divmod(sl, 2)
            out[b, c * CH : (c + 1) * CH] = (
                ffT[:, qb * 512 + qo * CH : qb * 512 + (qo + 1) * CH].T.astype(np.float32)
            )
    return out, res


def kernel(**inputs):
    out, _ = _run(inputs)
    return out


# revision 15
# speedup vs baseline: 1.1975x; 1.0439x over previous
"""Decoder block (single-head causal attention + GELU FFN) on 8 TRN2 NeuronCores.

Sharding: data parallel over batch (2 cores per batch), with the K AND V
projections token-split across the pair (each core projects its own half of
the sequence, then a pairwise AllGather shares both). Core c handles batch
b = c//2 and 1024 query tokens of that batch, chosen as four 256-token chunks
that balance the causal-attention workload:
  even cores (half 0): chunks 0, 3, 4, 7
  odd  cores (half 1): chunks 1, 2, 5, 6
The slot pairing makes the static k-tile counts per slot (4, 8, 12, 16) cover
both cores' needs with minimal waste (ideal is 36 tiles vs 40; the gap is
zeroed by the data-driven qpos mask). The SPMD program is identical on every
core; all per-core differences are data.

Performance structure (v2):
  - every matmul operand is fp16 (same PE rate as fp32r, half the DMA/SBUF)
  - V projection runs over the core's OWN half only (was: full sequence);
    K and V are exchanged by two pairwise AllGathers (K fires early, V later;
    P2 is restructured into scores-first/PV-second passes so the V gather
    latency hides behind all-slot score computation)
  - all weights / x tiles are multi-dim SBUF tiles filled by ONE or TWO big
    DMAs each (descriptor-generation on the issuing engine was costing
    ~630ns per 128KB tile; big transfers cut the issue count ~7x)
  - host pre-arranges every DRAM operand so each big DMA is contiguous per
    partition line (16KB runs)
  - Q/Wo/FFN matmuls interleave the two 512-token column blocks under one
    stationary weight load (halves LDWEIGHTS pressure; the pair partner's
    load hides under the 213ns FD=512 matmul)
  - P4 stages GELU results per 512-row block and stores once per block from
    the sync engine (64 -> 8 stores)
  - scalar engine runs only Identity in P1 and only Exp in P2 (activation
    table reloads cost 1.3us each)
"""

import numpy as np

D = 1024  # model dim
S = 2048  # sequence length
B = 4  # batch
M = 4096  # FFN dim
CH = 256  # q chunk (slot) size
NSLOT = 4  # q slots per core
NDT = D // 128  # 8 d-tiles
N_CORES = 8
NKT = [4, 8, 12, 16]  # k-tiles per slot (static max over the two paired cores)

_PROGRAM = None  # cached compiled program


def _build_program():
    import sys

    if "/opt/trn_rl_repo" not in sys.path:
        sys.path.insert(0, "/opt/trn_rl_repo")
    import concourse.bass as bass
    import concourse.tile as tile
    import concourse.mybir as mybir
    from concourse import bacc
    from concourse.bass import ts

    dt = mybir.dt
    AF = mybir.ActivationFunctionType
    ALU = mybir.AluOpType
    F32, F32R, F16 = dt.float32, dt.float32r, dt.float16

    nc = bacc.Bacc("TRN2", target_bir_lowering=False, debug=False, num_devices=8)

    # ---------------- DRAM I/O (all host-pre-arranged layouts) ----------------
    # weights: [128, i(8), 1024] with [p, i, c] = W.T[i*128+p, c]
    wqT = nc.dram_tensor("wqT", [128, NDT, D], F16, kind="ExternalInput").ap()
    wkT = nc.dram_tensor("wkT", [128, NDT, D], F16, kind="ExternalInput").ap()
    wvT = nc.dram_tensor("wvT", [128, NDT, D], F16, kind="ExternalInput").ap()
    woT = nc.dram_tensor("woT", [128, NDT, D], F16, kind="ExternalInput").ap()
    # FFN weight: [mb(8), 128, i(8), 512] with [mb, p, i, c] = Wf.T[i*128+p, mb*512+c]
    wfT = nc.dram_tensor("wfT", [M // 512, 128, NDT, 512], F16, kind="ExternalInput").ap()
    # x, own-half tokens in k order: [p, i, t] = x.T[i*128+p, half*1024+t]
    xaT = nc.dram_tensor("xaT", [128, NDT, S // 2], F16, kind="ExternalInput").ap()
    # x, own 4 chunks in q order
    xqT = nc.dram_tensor("xqT", [128, NDT, 4 * CH], F16, kind="ExternalInput").ap()
    bq = nc.dram_tensor("bq", [128, D // 128], F32, kind="ExternalInput").ap()
    bk = nc.dram_tensor("bk", [128, D // 128], F32, kind="ExternalInput").ap()
    bo2 = nc.dram_tensor("bo2", [128, D // 128], F32, kind="ExternalInput").ap()
    bfT = nc.dram_tensor("bfT", [128, M // 128], F32, kind="ExternalInput").ap()
    qpos = nc.dram_tensor("qpos", [1, 4 * CH], F32R, kind="ExternalInput").ap()
    iota_kt = nc.dram_tensor("iota_kt", [128, S // 128], F32, kind="ExternalInput").ap()
    # output: [mb(8), 128, mt*2+qb(8), 512] = ff^T[mb*512+mt*128+p, qb*512+q]
    ffT = nc.dram_tensor("ffT", [M // 512, 128, 8, 512], F16, kind="ExternalOutput").ap()

    with tile.TileContext(nc) as tc:
        with (
            tc.tile_pool(name="const", bufs=1) as cpool,
            tc.tile_pool(name="psum", bufs=1, space="PSUM") as pspool,
        ):
            # ---------------- constants (scalar engine issues these) --------
            ones_col_bf = cpool.tile([128, 1], F16, name="ones_col_bf", tag="ones_col_bf")
            nc.vector.memset(ones_col_bf[:], 1.0)
            ones_row_f = cpool.tile([1, 128], F32, name="ones_row_f", tag="ones_row_f")
            nc.vector.memset(ones_row_f[:], 1.0)
            ones_row = cpool.tile([1, 128], F32R, name="ones_row", tag="ones_row")
            nc.vector.tensor_copy(ones_row[:], ones_row_f[:])
            iota_sb = cpool.tile([128, S // 128], F32, name="iota", tag="iota")
            nc.scalar.dma_start(iota_sb[:], iota_kt[:])
            bq_sb = cpool.tile([128, D // 128], F32, name="bq", tag="bq")
            nc.scalar.dma_start(bq_sb[:], bq[:])
            bk_sb = cpool.tile([128, D // 128], F32, name="bk", tag="bk")
            nc.scalar.dma_start(bk_sb[:], bk[:])
            bo2_sb = cpool.tile([128, D // 128], F32, name="bo2", tag="bo2")
            nc.scalar.dma_start(bo2_sb[:], bo2[:])
            bf_sb = cpool.tile([128, M // 128], F32, name="bf", tag="bf")
            nc.scalar.dma_start(bf_sb[:], bfT[:])
            qpos_row = cpool.tile([1, 4 * CH], F32R, name="qpos_row", tag="qpos_row")
            nc.scalar.dma_start(qpos_row[:], qpos[:])
            qposB = cpool.tile([128, 4 * CH], F32, name="qposB", tag="qposB")

            # ------------- long-lived tiles: one pool spanning P1..P4 -------
            with (
                tc.tile_pool(name="main", bufs=1) as mp,
                tc.tile_pool(name="dram", bufs=1, space="DRAM") as dram,
            ):
                kT = mp.tile([128, NDT, S], F16, name="kT", tag="kT")
                vt = mp.tile([128, 16, D], F16, name="vt", tag="vt")
                wq_sb = mp.tile([128, NDT, D], F16, name="wq", tag="wq")
                wo_sb = mp.tile([128, NDT, D], F16, name="wo", tag="wo")
                xq = mp.tile([128, NDT, 4 * CH], F16, name="xq", tag="xq")
                qT = [
                    [mp.tile([128, 512], F16, name=f"qT{dt_}_{qb}", tag=f"qT{dt_}_{qb}") for qb in range(2)]
                    for dt_ in range(NDT)
                ]
                attnT = [
                    [mp.tile([128, 512], F16, name=f"at{dt_}_{qb}", tag=f"at{dt_}_{qb}") for qb in range(2)]
                    for dt_ in range(NDT)
                ]
                # DRAM bounce buffers for the pairwise K and V AllGathers.
                # Each projection is gathered in two 1MB halves so the
                # collectives fire earlier and finish well before P2 needs
                # the peer's tokens.
                ka_in = dram.tile([128, NDT, 512], F16, name="ka_in", tag="ka_in")
                ka_out = dram.tile([2, 128, NDT, 512], F16, name="ka_out", tag="ka_out")
                kb_in = dram.tile([128, NDT, 512], F16, name="kb_in", tag="kb_in")
                kb_out = dram.tile([2, 128, NDT, 512], F16, name="kb_out", tag="kb_out")
                va_in = dram.tile([128, 4, D], F16, name="va_in", tag="va_in")
                va_out = dram.tile([2, 128, 4, D], F16, name="va_out", tag="va_out")
                vb_in = dram.tile([128, 4, D], F16, name="vb_in", tag="vb_in")
                vb_out = dram.tile([2, 128, 4, D], F16, name="vb_out", tag="vb_out")

                def pair_gather(in_t, out_t):
                    nc.gpsimd.collective_compute(
                        "AllGather",
                        mybir.AluOpType.bypass,
                        replica_groups=[[0, 1], [2, 3], [4, 5], [6, 7]],
                        ins=[in_t[:].opt()],
                        outs=[out_t[:].opt()],
                    )

                # ---------------- P1 ----------------
                with tc.tile_pool(name="p1a", bufs=1) as p1a:
                    wk_sb = p1a.tile([128, NDT, D], F16, name="wk", tag="wk")
                    wv_sb = p1a.tile([128, NDT, D], F16, name="wv", tag="wv")
                    xa = p1a.tile([128, NDT, S // 2], F16, name="xa", tag="xa")

                    # DMA issue is tiered: all in-flight DMAs share wire
                    # bandwidth, so only the critical-path wk/xa stream starts
                    # immediately (per-i granularity: compute starts on the
                    # first 256KB). Later streams are gated on compute
                    # sentinels and issued from the otherwise-idle vector
                    # engine so they cannot steal bandwidth early.
                    nc.gpsimd.dma_start(wk_sb[:, 0, 0:512], wkT[:, 0, 0:512])
                    nc.sync.dma_start(xa[:, 0, 0:512], xaT[:, 0, 0:512])
                    nc.gpsimd.dma_start(wk_sb[:, 0, 512:1024], wkT[:, 0, 512:1024])
                    nc.sync.dma_start(xa[:, 0, 512:1024], xaT[:, 0, 512:1024])
                    for i in range(1, NDT):
                        nc.gpsimd.dma_start(wk_sb[:, i, :], wkT[:, i, :])
                        nc.sync.dma_start(xa[:, i, :], xaT[:, i, :])

                    # ---- K projection over own half (kT staged at [:, :, 0:1024])
                    for tb in range(2):
                        for og in range(2):
                            ps4 = [
                                pspool.tile([128, 512], F32, name=f"at{j}", tag=f"at{j}", bufs=1)
                                for j in range(4)
                            ]
                            for i in range(NDT):
                                for j in range(4):
                                    nc.tensor.matmul(
                                        ps4[j][:],
                                        wk_sb[:, i, ts(og * 4 + j, 128)],
                                        xa[:, i, ts(tb, 512)],
                                        start=(i == 0), stop=(i == NDT - 1),
                                    )
                            for j in range(4):
                                ot = og * 4 + j
                                nc.scalar.activation(
                                    kT[:, ot, ts(tb, 512)], ps4[j][:], AF.Identity,
                                    bias=bk_sb[:, ot : ot + 1],
                                )
                            if tb == 0 and og == 0:
                                # tier-1 issue: scalar reaches here only after
                                # og0's drains, so wv/xq cannot steal wire
                                # bandwidth from the critical wk/xa stream
                                for h in range(2):
                                    nc.scalar.dma_start(
                                        wv_sb[:, ts(h, 4), :], wvT[:, ts(h, 4), :]
                                    )
                                nc.scalar.dma_start(xq[:], xqT[:])
                        # stage this token block and fire its K gather
                        kin = ka_in if tb == 0 else kb_in
                        nc.sync.dma_start(kin[:], kT[:, :, ts(tb, 512)])
                        pair_gather(kin, ka_out if tb == 0 else kb_out)

                    # ---- V projection over own half (token-major, 2 banks/tt)
                    for tb in range(2):
                        for tt in range(4):
                            ps2 = [
                                pspool.tile([128, 512], F32, name=f"mm{ob}", tag="mm", bufs=4)
                                for ob in range(2)
                            ]
                            for i in range(NDT):
                                for ob in range(2):
                                    nc.tensor.matmul(
                                        ps2[ob][:],
                                        xa[:, i, tb * 512 + tt * 128 : tb * 512 + (tt + 1) * 128],
                                        wv_sb[:, i, ts(ob, 512)],
                                        start=(i == 0), stop=(i == NDT - 1),
                                    )
                            for ob in range(2):
                                nc.scalar.activation(
                                    vt[:, tb * 4 + tt, ts(ob, 512)], ps2[ob][:], AF.Identity
                                )
                            if tb == 0 and tt == 0:
                                # tier-2 issue (see tier-1 note)
                                for h in range(2):
                                    nc.scalar.dma_start(
                                        wq_sb[:, ts(h, 4), :], wqT[:, ts(h, 4), :]
                                    )
                        # stage this V token block and fire its gather.
                        # vt[:, 0:8] doubles as the projection scratch; the
                        # readback overwrites all 16 slots in token order.
                        vin = va_in if tb == 0 else vb_in
                        nc.scalar.dma_start(vin[:], vt[:, ts(tb, 4), :])
                        pair_gather(vin, va_out if tb == 0 else vb_out)

                    # qpos broadcast (fills the gather window)
                    for i in range(4 * CH // 512):
                        bc_ps = pspool.tile([128, 512], F32, name="small", tag="at0", bufs=1)
                        nc.tensor.matmul(
                            bc_ps[:], ones_row[:], qpos_row[:, ts(i, 512)],
                            start=True, stop=True,
                        )
                        nc.scalar.activation(qposB[:, ts(i, 512)], bc_ps[:], AF.Identity)

                    # ---- Q projection: qb pair interleaved under one weight
                    for ot in range(NDT):
                        psq = [
                            pspool.tile([128, 512], F32, name=f"mmq{qb}", tag="mm", bufs=4)
                            for qb in range(2)
                        ]
                        for i in range(NDT):
                            for qb in range(2):
                                nc.tensor.matmul(
                                    psq[qb][:],
                                    wq_sb[:, i, ts(ot, 128)],
                                    xq[:, i, ts(qb, 512)],
                                    start=(i == 0), stop=(i == NDT - 1),
                                )
                        for qb in range(2):
                            nc.scalar.activation(
                                qT[ot][qb][:], psq[qb][:], AF.Identity,
                                bias=bq_sb[:, ot : ot + 1],
                            )
                        if ot == 0:
                            # tier-3: wo is only needed in P3; issuing it this
                            # late keeps it off the K/V gather's wire window
                            for h in range(2):
                                nc.scalar.dma_start(
                                    wo_sb[:, ts(h, 4), :], woT[:, ts(h, 4), :]
                                )

                # gathered K^T / V readback in true token order, ordered by
                # first use in P2 (sync engine; waits ride on the collectives)
                for r in range(2):
                    nc.sync.dma_start(
                        kT[:, :, r * 1024 + 0 : r * 1024 + 512], ka_out[r]
                    )
                    nc.sync.dma_start(
                        kT[:, :, r * 1024 + 512 : r * 1024 + 1024], kb_out[r]
                    )
                for r in range(2):
                    nc.sync.dma_start(vt[:, r * 8 + 0 : r * 8 + 4, :], va_out[r])
                    nc.sync.dma_start(vt[:, r * 8 + 4 : r * 8 + 8, :], vb_out[r])

                # ---------------- P2: attention ----------------
                # Slots are processed as PAIRS sharing a 512-token q block:
                # the k-range both slots need runs at FD=512 (LDWEIGHTS fully
                # hidden), the hi-slot's excess k-tiles at FD=256.
                # pass A computes scores+exp+mask+denom for all slots first
                # (probs stay resident), so the V gather hides behind it.
                with tc.tile_pool(name="p2", bufs=1) as p2:
                    LOHI = [(NKT[0], NKT[1]), (NKT[2], NKT[3])]
                    pt_sh = [
                        [
                            p2.tile([128, 512], F16, name=f"pts{p}_{k}", tag=f"pts{p}_{k}", bufs=1)
                            for k in range(LOHI[p][0])
                        ]
                        for p in range(2)
                    ]
                    pt_ex = [
                        [
                            p2.tile([128, CH], F16, name=f"ptx{p}_{j}", tag=f"ptx{p}_{j}", bufs=1)
                            for j in range(LOHI[p][1] - LOHI[p][0])
                        ]
                        for p in range(2)
                    ]
                    recipB = [
                        p2.tile([128, CH], F32, name=f"recipB{sl}", tag=f"recipB{sl}", bufs=1)
                        for sl in range(NSLOT)
                    ]
                    for p in range(2):
                        lo, hi = LOHI[p]
                        # shared k-range: FD=512 over both slots
                        for k in range(lo):
                            ps = pspool.tile([128, 512], F32, name="mm", tag="mm", bufs=4)
                            for i in range(NDT):
                                nc.tensor.matmul(
                                    ps[:],
                                    kT[:, i, ts(k, 128)],
                                    qT[i][p][:],
                                    start=(i == 0), stop=(i == NDT - 1),
                                )
                            nc.scalar.activation(
                                pt_sh[p][k][:], ps[:], AF.Exp, scale=1.0 / 32.0
                            )
                            if k >= lo - 4:
                                # mask applies to the lo slot's columns only
                                msk = p2.tile([128, CH], F16, name="msk", tag="msk", bufs=2)
                                nc.vector.tensor_scalar(
                                    out=msk[:],
                                    in0=qposB[:, ts(2 * p, CH)],
                                    scalar1=iota_sb[:, k : k + 1],
                                    scalar2=None,
                                    op0=ALU.is_ge,
                                )
                                nc.vector.tensor_tensor(
                                    out=pt_sh[p][k][:, 0:CH],
                                    in0=pt_sh[p][k][:, 0:CH], in1=msk[:],
                                    op=ALU.mult,
                                )
                        # excess k-tiles: hi slot only, FD=256, all masked
                        for j, k in enumerate(range(lo, hi)):
                            ps = pspool.tile([128, CH], F32, name="mm", tag="mm", bufs=4)
                            for i in range(NDT):
                                nc.tensor.matmul(
                                    ps[:],
                                    kT[:, i, ts(k, 128)],
                                    qT[i][p][:, CH : 2 * CH],
                                    start=(i == 0), stop=(i == NDT - 1),
                                )
                            praw = p2.tile([128, CH], F16, name="praw", tag="praw", bufs=2)
                            nc.scalar.activation(
                                praw[:], ps[:], AF.Exp, scale=1.0 / 32.0
                            )
                            msk = p2.tile([128, CH], F16, name="msk", tag="msk", bufs=2)
                            nc.vector.tensor_scalar(
                                out=msk[:],
                                in0=qposB[:, ts(2 * p + 1, CH)],
                                scalar1=iota_sb[:, k : k + 1],
                                scalar2=None,
                                op0=ALU.is_ge,
                            )
                            nc.vector.tensor_tensor(
                                out=pt_ex[p][j][:], in0=praw[:], in1=msk[:],
                                op=ALU.mult,
                            )
                        # denominators per slot (FD=256 chains), then recip
                        for h_ in range(2):
                            sl = 2 * p + h_
                            dn_ps = pspool.tile([1, CH], F32, name="small", tag="at0", bufs=1)
                            nmm = LOHI[p][0] if h_ == 0 else hi
                            kk = 0
                            for k in range(lo):
                                nc.tensor.matmul(
                                    dn_ps[:], ones_col_bf[:],
                                    pt_sh[p][k][:, ts(h_, CH)],
                                    start=(kk == 0), stop=(kk == nmm - 1),
                                )
                                kk += 1
                                if h_ == 0 and kk == nmm:
                                    break
                            if h_ == 1:
                                for j in range(hi - lo):
                                    nc.tensor.matmul(
                                        dn_ps[:], ones_col_bf[:], pt_ex[p][j][:],
                                        start=(kk == 0), stop=(kk == nmm - 1),
                                    )
                                    kk += 1
                            dn_r = p2.tile([1, CH], F32R, name="dn_r", tag="dn_r", bufs=2)
                            nc.vector.tensor_copy(dn_r[:], dn_ps[:])
                            rb_ps = pspool.tile([128, CH], F32, name="small2", tag="at1", bufs=1)
                            nc.tensor.matmul(
                                rb_ps[:], ones_row[:], dn_r[:], start=True, stop=True
                            )
                            nc.vector.reciprocal(recipB[sl][:], rb_ps[:])

                    # pass B: attn^T = (P @ V)^T scaled by 1/denom
                    for p in range(2):
                        lo, hi = LOHI[p]
                        for half in range(2):
                            for d4 in range(4):
                                d_ = half * 4 + d4
                                aps = pspool.tile([128, 512], F32, name=f"at{d4}", tag=f"at{d4}", bufs=1)
                                for k in range(lo):
                                    nc.tensor.matmul(
                                        aps[:],
                                        vt[:, k, ts(d_, 128)],
                                        pt_sh[p][k][:],
                                        start=(k == 0), stop=False,
                                        skip_group_check=True,
                                    )
                                for j, k in enumerate(range(lo, hi)):
                                    nc.tensor.matmul(
                                        aps[:, CH : 2 * CH],
                                        vt[:, k, ts(d_, 128)],
                                        pt_ex[p][j][:],
                                        start=False, stop=(k == hi - 1),
                                        skip_group_check=True,
                                    )
                                if hi == lo:
                                    pass
                                for h_ in range(2):
                                    nc.vector.tensor_tensor(
                                        out=attnT[d_][p][:, ts(h_, CH)],
                                        in0=aps[:, ts(h_, CH)],
                                        in1=recipB[2 * p + h_][:],
                                        op=ALU.mult,
                                    )

                # ---------------- P3 + P4 ----------------
                with tc.tile_pool(name="p34", bufs=1) as p34:
                    outT = [
                        [p34.tile([128, 512], F16, name=f"oT{dt_}_{qb}", tag=f"oT{dt_}_{qb}") for qb in range(2)]
                        for dt_ in range(NDT)
                    ]
                    # P3: qb pair interleaved under one Wo weight block
                    for ot in range(NDT):
                        pso = [
                            pspool.tile([128, 512], F32, name=f"mmo{qb}", tag="mm", bufs=4)
                            for qb in range(2)
                        ]
                        for i in range(NDT):
                            for qb in range(2):
                                nc.tensor.matmul(
                                    pso[qb][:],
                                    wo_sb[:, i, ts(ot, 128)],
                                    attnT[i][qb][:],
                                    start=(i == 0), stop=(i == NDT - 1),
                                )
                        for qb in range(2):
                            nc.scalar.activation(
                                outT[ot][qb][:], pso[qb][:], AF.Identity,
                                bias=bo2_sb[:, ot : ot + 1],
                            )

                    # P4: FFN + GELU; one weight DMA and one store per mb block
                    for mb in range(M // 512):
                        wfb = p34.tile([128, NDT, 512], F16, name="wfb", tag="wfb", bufs=2)
                        nc.gpsimd.dma_start(wfb[:], wfT[mb])
                        for mt in range(4):
                            m = mb * 4 + mt
                            psf = [
                                pspool.tile([128, 512], F32, name=f"mmf{qb}", tag="mm", bufs=4)
                                for qb in range(2)
                            ]
                            for i in range(NDT):
                                for qb in range(2):
                                    nc.tensor.matmul(
                                        psf[qb][:],
                                        wfb[:, i, ts(mt, 128)],
                                        outT[i][qb][:],
                                        start=(i == 0), stop=(i == NDT - 1),
                                    )
                            st = p34.tile([128, 2, 512], F16, name="ffstage", tag="ffstage", bufs=4)
                            for qb in range(2):
                                nc.scalar.activation(
                                    st[:, qb, :], psf[qb][:], AF.Gelu,
                                    bias=bf_sb[:, m : m + 1],
                                )
                            # fine-grained stores: the last tile's store tail
                            # is 256KB, not 1MB
                            nc.sync.dma_start(ffT[mb, :, ts(mt, 2), :], st[:])

    nc.compile()
    return nc


def _get_program():
    global _PROGRAM
    if _PROGRAM is None:
        _PROGRAM = _build_program()
    return _PROGRAM


def _owned_chunks(core):
    """The four 256-token chunk indices this core owns, in slot order."""
    if core % 2 == 0:
        return (0, 3, 4, 7)
    return (1, 2, 5, 6)


def _blocked(a):
    """[1024, W] -> [128, 8, W] with [p, i, c] = a[i*128+p, c]."""
    W = a.shape[1]
    return np.ascontiguousarray(a.reshape(8, 128, W).transpose(1, 0, 2))


def _make_in_maps(x, Wq, bq, Wk, bk, Wv, bv, Wo, bo, Wf, bf):
    f32, f16 = np.float32, np.float16
    wqT = _blocked(np.asarray(Wq.T, dtype=f16))
    wkT = _blocked(np.asarray(Wk.T, dtype=f16))
    wvT = _blocked(np.asarray(Wv.T, dtype=f16))
    woT = _blocked(np.asarray(Wo.T, dtype=f16))
    # wfT[mb, p, i, c] = Wf.T[i*128+p, mb*512+c]
    wfT = np.ascontiguousarray(
        np.asarray(Wf.T, dtype=f16).reshape(8, 128, 8, 512).transpose(2, 1, 0, 3)
    )
    bo2 = (Wo.astype(np.float64) @ bv.astype(np.float64) + bo.astype(np.float64))
    bo2 = np.ascontiguousarray(bo2.astype(f32).reshape(D // 128, 128).T)
    bfT = np.ascontiguousarray(bf.reshape(M // 128, 128).T, dtype=f32)
    iota = (
        np.arange(128, dtype=f32)[:, None]
        + 128.0 * np.arange(S // 128, dtype=f32)[None, :]
    )
    shared = {
        "wqT": wqT, "wkT": wkT, "wvT": wvT, "woT": woT, "wfT": wfT,
        "bq": np.ascontiguousarray(bq.reshape(D // 128, 128).T, dtype=f32),
        "bk": np.ascontiguousarray(bk.reshape(D // 128, 128).T, dtype=f32),
        "bo2": bo2,
        "bfT": bfT,
        "iota_kt": np.ascontiguousarray(iota),
    }
    in_maps = []
    for core in range(N_CORES):
        b = core // 2
        chunks = _owned_chunks(core)
        xTb = np.asarray(x[b].T, dtype=f16)  # [D, S]
        half = core % 2  # rank within the pair: rank0 owns tokens 0:S/2
        xaT = _blocked(xTb[:, half * (S // 2) : (half + 1) * (S // 2)])
        xqT = _blocked(
            np.concatenate([xTb[:, c * CH : (c + 1) * CH] for c in chunks], axis=1)
        )
        qp = np.concatenate(
            [np.arange(c * CH, (c + 1) * CH) for c in chunks]
        ).astype(f32)[None, :]
        in_maps.append(
            {**shared, "xaT": xaT, "xqT": xqT,
             "qpos": np.ascontiguousarray(qp)}
        )
    return in_maps


def _run(inputs, trace=False, trace_cores=None, tmpdir=None):
    import sys

    if "/opt/trn_rl_repo" not in sys.path:
        sys.path.insert(0, "/opt/trn_rl_repo")
    from concourse.bass_utils import run_bass_kernel_spmd

    nc = _get_program()
    in_maps = _make_in_maps(**inputs)
    res = run_bass_kernel_spmd(
        nc, in_maps, list(range(N_CORES)), trace=trace, trace_cores=trace_cores,
        tmpdir=tmpdir,
    )
    out = np.empty((B, S, M), dtype=np.float32)
    for core in range(N_CORES):
        b = core // 2
        chunks = _owned_chunks(core)
        # ffT[mb, p, mt*2+qb, q] = ff^T[mb*512+mt*128+p, qb*512+q]
        raw = res.results[core]["ffT"].reshape(8, 128, 4, 2, 512)
        ffT = np.ascontiguousarray(
            raw.transpose(0, 2, 1, 3, 4)
        ).reshape(M, 4 * CH)
        for sl, c in enumerate(chunks):
            qb, qo = divmod(sl, 2)
            out[b, c * CH : (c + 1) * CH] = (
                ffT[:, qb * 512 + qo * CH : qb * 512 + (qo + 1) * CH].T.astype(np.float32)
            )
    return out, res


def kernel(**inputs):
    out, _ = _run(inputs)
    return out


# revision 19
# speedup vs baseline: 1.2217x; 1.0202x over previous
"""Decoder block (single-head causal attention + GELU FFN) on 8 TRN2 NeuronCores.

Sharding: data parallel over batch (2 cores per batch), with the K AND V
projections token-split across the pair (each core projects its own half of
the sequence, then a pairwise AllGather shares both). Core c handles batch
b = c//2 and 1024 query tokens of that batch, chosen as four 256-token chunks
that balance the causal-attention workload:
  even cores (half 0): chunks 0, 3, 4, 7
  odd  cores (half 1): chunks 1, 2, 5, 6
The slot pairing makes the static k-tile counts per slot (4, 8, 12, 16) cover
both cores' needs with minimal waste (ideal is 36 tiles vs 40; the gap is
zeroed by the data-driven qpos mask). The SPMD program is identical on every
core; all per-core differences are data.

Performance structure (v2):
  - every matmul operand is fp16 (same PE rate as fp32r, half the DMA/SBUF)
  - V projection runs over the core's OWN half only (was: full sequence);
    K and V are exchanged by two pairwise AllGathers (K fires early, V later;
    P2 is restructured into scores-first/PV-second passes so the V gather
    latency hides behind all-slot score computation)
  - all weights / x tiles are multi-dim SBUF tiles filled by ONE or TWO big
    DMAs each (descriptor-generation on the issuing engine was costing
    ~630ns per 128KB tile; big transfers cut the issue count ~7x)
  - host pre-arranges every DRAM operand so each big DMA is contiguous per
    partition line (16KB runs)
  - Q/Wo/FFN matmuls interleave the two 512-token column blocks under one
    stationary weight load (halves LDWEIGHTS pressure; the pair partner's
    load hides under the 213ns FD=512 matmul)
  - P4 stages GELU results per 512-row block and stores once per block from
    the sync engine (64 -> 8 stores)
  - scalar engine runs only Identity in P1 and only Exp in P2 (activation
    table reloads cost 1.3us each)
"""

import numpy as np

D = 1024  # model dim
S = 2048  # sequence length
B = 4  # batch
M = 4096  # FFN dim
CH = 256  # q chunk (slot) size
NSLOT = 4  # q slots per core
NDT = D // 128  # 8 d-tiles
N_CORES = 8
NKT = [4, 8, 12, 16]  # k-tiles per slot (static max over the two paired cores)

_PROGRAM = None  # cached compiled program


def _build_program():
    import sys

    if "/opt/trn_rl_repo" not in sys.path:
        sys.path.insert(0, "/opt/trn_rl_repo")
    import concourse.bass as bass
    import concourse.tile as tile
    import concourse.mybir as mybir
    from concourse import bacc
    from concourse.bass import ts

    dt = mybir.dt
    AF = mybir.ActivationFunctionType
    ALU = mybir.AluOpType
    F32, F32R, F16, F8 = dt.float32, dt.float32r, dt.float16, dt.float8e4
    DR = mybir.MatmulPerfMode.DoubleRow

    nc = bacc.Bacc("TRN2", target_bir_lowering=False, debug=False, num_devices=8)

    # ---------------- DRAM I/O (all host-pre-arranged layouts) ----------------
    # weights: [128, i(8), 1024] with [p, i, c] = W.T[i*128+p, c]
    wqT = nc.dram_tensor("wqT", [128, NDT, D], F16, kind="ExternalInput").ap()
    wkT = nc.dram_tensor("wkT", [128, NDT, D], F16, kind="ExternalInput").ap()
    wvT = nc.dram_tensor("wvT", [128, NDT, D], F16, kind="ExternalInput").ap()
    woT = nc.dram_tensor("woT", [128, NDT, D], F16, kind="ExternalInput").ap()
    # FFN weight: [mb(8), 128, i(8), 512] with [mb, p, i, c] = Wf.T[i*128+p, mb*512+c]
    wfT = nc.dram_tensor("wfT", [M // 512, 128, NDT, 512], F16, kind="ExternalInput").ap()
    # x, own-half tokens in k order: [p, i, t] = x.T[i*128+p, half*1024+t]
    xaT = nc.dram_tensor("xaT", [128, NDT, S // 2], F16, kind="ExternalInput").ap()
    # x, own 4 chunks in q order
    xqT = nc.dram_tensor("xqT", [128, NDT, 4 * CH], F16, kind="ExternalInput").ap()
    bq = nc.dram_tensor("bq", [128, D // 128], F32, kind="ExternalInput").ap()
    bk = nc.dram_tensor("bk", [128, D // 128], F32, kind="ExternalInput").ap()
    bo2 = nc.dram_tensor("bo2", [128, D // 128], F32, kind="ExternalInput").ap()
    bfT = nc.dram_tensor("bfT", [128, M // 128], F32, kind="ExternalInput").ap()
    qpos = nc.dram_tensor("qpos", [1, 4 * CH], F32R, kind="ExternalInput").ap()
    bf_row = nc.dram_tensor("bf_row", [1, M], F32R, kind="ExternalInput").ap()
    iota_kt = nc.dram_tensor("iota_kt", [128, S // 128], F32, kind="ExternalInput").ap()
    # output: [mb(8), 128p, t8(8), 512c] = ff[(t8//4)*512+(t8%4)*128+p, mb*512+c]
    ffT = nc.dram_tensor("ffT", [M // 512, 128, 8, 512], F16, kind="ExternalOutput").ap()

    with tile.TileContext(nc) as tc:
        with (
            tc.tile_pool(name="const", bufs=1) as cpool,
            tc.tile_pool(name="psum", bufs=1, space="PSUM") as pspool,
        ):
            # ---------------- constants (scalar engine issues these) --------
            ones_col_bf = cpool.tile([128, 1], F16, name="ones_col_bf", tag="ones_col_bf")
            nc.vector.memset(ones_col_bf[:], 1.0)
            ones_row_f = cpool.tile([1, 128], F32, name="ones_row_f", tag="ones_row_f")
            nc.vector.memset(ones_row_f[:], 1.0)
            ones_row = cpool.tile([1, 128], F32R, name="ones_row", tag="ones_row")
            nc.vector.tensor_copy(ones_row[:], ones_row_f[:])
            iota_sb = cpool.tile([128, S // 128], F32, name="iota", tag="iota")
            nc.scalar.dma_start(iota_sb[:], iota_kt[:])
            bq_sb = cpool.tile([128, D // 128], F32, name="bq", tag="bq")
            nc.scalar.dma_start(bq_sb[:], bq[:])
            bk_sb = cpool.tile([128, D // 128], F32, name="bk", tag="bk")
            nc.scalar.dma_start(bk_sb[:], bk[:])
            bo2_sb = cpool.tile([128, D // 128], F32, name="bo2", tag="bo2")
            nc.scalar.dma_start(bo2_sb[:], bo2[:])
            bf_sb = cpool.tile([128, M // 128], F32, name="bf", tag="bf")
            nc.scalar.dma_start(bf_sb[:], bfT[:])
            qpos_row = cpool.tile([1, 4 * CH], F32R, name="qpos_row", tag="qpos_row")
            nc.scalar.dma_start(qpos_row[:], qpos[:])
            bfr_sb = cpool.tile([1, M], F32R, name="bfr_sb", tag="bfr_sb")
            nc.scalar.dma_start(bfr_sb[:], bf_row[:])
            qposB = cpool.tile([128, 4 * CH], F32, name="qposB", tag="qposB")

            # ------------- long-lived tiles: one pool spanning P1..P4 -------
            with (
                tc.tile_pool(name="main", bufs=1) as mp,
                tc.tile_pool(name="dram", bufs=1, space="DRAM") as dram,
            ):
                # fp8 K^T in DoubleRow pair layout: [p, i2, s, tok],
                # contraction d = (2*i2+s)*128 + p
                kT8 = mp.tile([128, 4, 2, S], F8, name="kT8", tag="kT8")
                vt = mp.tile([128, 16, D], F16, name="vt", tag="vt")
                wq_sb = mp.tile([128, NDT, D], F16, name="wq", tag="wq")
                wo_sb = mp.tile([128, NDT, D], F16, name="wo", tag="wo")
                xq = mp.tile([128, NDT, 4 * CH], F16, name="xq", tag="xq")
                qT8 = [
                    mp.tile([128, 4, 2, 512], F8, name=f"qT8_{qb}", tag=f"qT8_{qb}")
                    for qb in range(2)
                ]
                attnT = [
                    [mp.tile([128, 512], F16, name=f"at{dt_}_{qb}", tag=f"at{dt_}_{qb}") for qb in range(2)]
                    for dt_ in range(NDT)
                ]
                # DRAM bounce buffers for the pairwise K and V AllGathers.
                # Each projection is gathered in two 1MB halves so the
                # collectives fire earlier and finish well before P2 needs
                # the peer's tokens.
                ka_in = dram.tile([128, 4, 2, 512], F8, name="ka_in", tag="ka_in")
                ka_out = dram.tile([2, 128, 4, 2, 512], F8, name="ka_out", tag="ka_out")
                kb_in = dram.tile([128, 4, 2, 512], F8, name="kb_in", tag="kb_in")
                kb_out = dram.tile([2, 128, 4, 2, 512], F8, name="kb_out", tag="kb_out")
                va_in = dram.tile([128, 4, D], F16, name="va_in", tag="va_in")
                va_out = dram.tile([2, 128, 4, D], F16, name="va_out", tag="va_out")
                vb_in = dram.tile([128, 4, D], F16, name="vb_in", tag="vb_in")
                vb_out = dram.tile([2, 128, 4, D], F16, name="vb_out", tag="vb_out")

                def pair_gather(in_t, out_t):
                    nc.gpsimd.collective_compute(
                        "AllGather",
                        mybir.AluOpType.bypass,
                        replica_groups=[[0, 1], [2, 3], [4, 5], [6, 7]],
                        ins=[in_t[:].opt()],
                        outs=[out_t[:].opt()],
                    )

                # ---------------- P1 ----------------
                with tc.tile_pool(name="p1a", bufs=1) as p1a:
                    wk_sb = p1a.tile([128, NDT, D], F16, name="wk", tag="wk")
                    wv_sb = p1a.tile([128, NDT, D], F16, name="wv", tag="wv")
                    xa = p1a.tile([128, NDT, S // 2], F16, name="xa", tag="xa")

                    # DMA issue is tiered: all in-flight DMAs share wire
                    # bandwidth, so only the critical-path wk/xa stream starts
                    # immediately (per-i granularity: compute starts on the
                    # first 256KB). Later streams are gated on compute
                    # sentinels and issued from the otherwise-idle vector
                    # engine so they cannot steal bandwidth early.
                    nc.gpsimd.dma_start(wk_sb[:, 0, 0:512], wkT[:, 0, 0:512])
                    nc.sync.dma_start(xa[:, 0, 0:512], xaT[:, 0, 0:512])
                    nc.gpsimd.dma_start(wk_sb[:, 0, 512:1024], wkT[:, 0, 512:1024])
                    nc.sync.dma_start(xa[:, 0, 512:1024], xaT[:, 0, 512:1024])
                    for i in range(1, NDT):
                        nc.gpsimd.dma_start(wk_sb[:, i, :], wkT[:, i, :])
                        nc.sync.dma_start(xa[:, i, :], xaT[:, i, :])

                    # ---- K projection over own half (kT staged at [:, :, 0:1024])
                    for tb in range(2):
                        for og in range(2):
                            ps4 = [
                                pspool.tile([128, 512], F32, name=f"at{j}", tag=f"at{j}", bufs=1)
                                for j in range(4)
                            ]
                            for i in range(NDT):
                                for j in range(4):
                                    nc.tensor.matmul(
                                        ps4[j][:],
                                        wk_sb[:, i, ts(og * 4 + j, 128)],
                                        xa[:, i, ts(tb, 512)],
                                        start=(i == 0), stop=(i == NDT - 1),
                                    )
                            for j in range(4):
                                ot = og * 4 + j
                                nc.scalar.activation(
                                    kT8[:, ot // 2, ot % 2, ts(tb, 512)], ps4[j][:],
                                    AF.Identity, bias=bk_sb[:, ot : ot + 1],
                                )
                            if tb == 0 and og == 0:
                                # tier-1 issue: scalar reaches here only after
                                # og0's drains, so wv/xq cannot steal wire
                                # bandwidth from the critical wk/xa stream
                                for h in range(2):
                                    nc.scalar.dma_start(
                                        wv_sb[:, ts(h, 4), :], wvT[:, ts(h, 4), :]
                                    )
                                nc.scalar.dma_start(xq[:], xqT[:])
                        # stage this token block and fire its K gather
                        kin = ka_in if tb == 0 else kb_in
                        nc.sync.dma_start(kin[:], kT8[:, :, :, ts(tb, 512)])
                        pair_gather(kin, ka_out if tb == 0 else kb_out)

                    # ---- V projection over own half (token-major, 2 banks/tt)
                    for tb in range(2):
                        for tt in range(4):
                            ps2 = [
                                pspool.tile([128, 512], F32, name=f"mm{ob}", tag="mm", bufs=4)
                                for ob in range(2)
                            ]
                            for i in range(NDT):
                                for ob in range(2):
                                    nc.tensor.matmul(
                                        ps2[ob][:],
                                        xa[:, i, tb * 512 + tt * 128 : tb * 512 + (tt + 1) * 128],
                                        wv_sb[:, i, ts(ob, 512)],
                                        start=(i == 0), stop=(i == NDT - 1),
                                    )
                            for ob in range(2):
                                nc.scalar.activation(
                                    vt[:, tb * 4 + tt, ts(ob, 512)], ps2[ob][:], AF.Identity
                                )
                            if tb == 0 and tt == 0:
                                # tier-2 issue (see tier-1 note)
                                for h in range(2):
                                    nc.scalar.dma_start(
                                        wq_sb[:, ts(h, 4), :], wqT[:, ts(h, 4), :]
                                    )
                        # stage this V token block and fire its gather.
                        # vt[:, 0:8] doubles as the projection scratch; the
                        # readback overwrites all 16 slots in token order.
                        vin = va_in if tb == 0 else vb_in
                        nc.scalar.dma_start(vin[:], vt[:, ts(tb, 4), :])
                        pair_gather(vin, va_out if tb == 0 else vb_out)

                    # qpos broadcast (fills the gather window)
                    for i in range(4 * CH // 512):
                        bc_ps = pspool.tile([128, 512], F32, name="small", tag="at0", bufs=1)
                        nc.tensor.matmul(
                            bc_ps[:], ones_row[:], qpos_row[:, ts(i, 512)],
                            start=True, stop=True,
                        )
                        nc.scalar.activation(qposB[:, ts(i, 512)], bc_ps[:], AF.Identity)

                    # ---- Q projection: qb pair interleaved under one weight
                    for ot in range(NDT):
                        psq = [
                            pspool.tile([128, 512], F32, name=f"mmq{qb}", tag="mm", bufs=4)
                            for qb in range(2)
                        ]
                        for i in range(NDT):
                            for qb in range(2):
                                nc.tensor.matmul(
                                    psq[qb][:],
                                    wq_sb[:, i, ts(ot, 128)],
                                    xq[:, i, ts(qb, 512)],
                                    start=(i == 0), stop=(i == NDT - 1),
                                )
                        for qb in range(2):
                            nc.scalar.activation(
                                qT8[qb][:, ot // 2, ot % 2, :], psq[qb][:],
                                AF.Identity, bias=bq_sb[:, ot : ot + 1],
                            )
                        if ot == 0:
                            # tier-3: wo is only needed in P3; issuing it this
                            # late keeps it off the K/V gather's wire window
                            for h in range(2):
                                nc.scalar.dma_start(
                                    wo_sb[:, ts(h, 4), :], woT[:, ts(h, 4), :]
                                )

                # gathered K^T / V readback in true token order, ordered by
                # first use in P2 (sync engine; waits ride on the collectives)
                for r in range(2):
                    nc.sync.dma_start(
                        kT8[:, :, :, r * 1024 + 0 : r * 1024 + 512], ka_out[r]
                    )
                    nc.sync.dma_start(
                        kT8[:, :, :, r * 1024 + 512 : r * 1024 + 1024], kb_out[r]
                    )
                for r in range(2):
                    nc.sync.dma_start(vt[:, r * 8 + 0 : r * 8 + 4, :], va_out[r])
                    nc.sync.dma_start(vt[:, r * 8 + 4 : r * 8 + 8, :], vb_out[r])

                # ---------------- P2: attention ----------------
                # Slots are processed as PAIRS sharing a 512-token q block:
                # the k-range both slots need runs at FD=512 (LDWEIGHTS fully
                # hidden), the hi-slot's excess k-tiles at FD=256.
                # pass A computes scores+exp+mask+denom for all slots first
                # (probs stay resident), so the V gather hides behind it.
                with tc.tile_pool(name="p2", bufs=1) as p2:
                    LOHI = [(NKT[0], NKT[1]), (NKT[2], NKT[3])]
                    pt_sh = [
                        [
                            p2.tile([128, 512], F16, name=f"pts{p}_{k}", tag=f"pts{p}_{k}", bufs=1)
                            for k in range(LOHI[p][0])
                        ]
                        for p in range(2)
                    ]
                    pt_ex = [
                        [
                            p2.tile([128, CH], F16, name=f"ptx{p}_{j}", tag=f"ptx{p}_{j}", bufs=1)
                            for j in range(LOHI[p][1] - LOHI[p][0])
                        ]
                        for p in range(2)
                    ]
                    recipB = [
                        p2.tile([128, CH], F32, name=f"recipB{sl}", tag=f"recipB{sl}", bufs=1)
                        for sl in range(NSLOT)
                    ]
                    for p in range(2):
                        lo, hi = LOHI[p]
                        # shared k-range: FD=512 over both slots
                        for k in range(lo):
                            ps = pspool.tile([128, 512], F32, name="mm", tag="mm", bufs=4)
                            for i2 in range(4):
                                nc.tensor.matmul(
                                    ps[:],
                                    kT8[:, i2, :, ts(k, 128)],
                                    qT8[p][:, i2, :, :],
                                    start=(i2 == 0), stop=(i2 == 3),
                                    perf_mode=DR,
                                )
                            nc.scalar.activation(
                                pt_sh[p][k][:], ps[:], AF.Exp, scale=1.0 / 32.0
                            )
                            if k >= lo - 4:
                                # mask applies to the lo slot's columns only
                                msk = p2.tile([128, CH], F16, name="msk", tag="msk", bufs=2)
                                nc.vector.tensor_scalar(
                                    out=msk[:],
                                    in0=qposB[:, ts(2 * p, CH)],
                                    scalar1=iota_sb[:, k : k + 1],
                                    scalar2=None,
                                    op0=ALU.is_ge,
                                )
                                nc.vector.tensor_tensor(
                                    out=pt_sh[p][k][:, 0:CH],
                                    in0=pt_sh[p][k][:, 0:CH], in1=msk[:],
                                    op=ALU.mult,
                                )
                        # excess k-tiles: hi slot only, FD=256, all masked
                        for j, k in enumerate(range(lo, hi)):
                            ps = pspool.tile([128, CH], F32, name="mm", tag="mm", bufs=4)
                            for i2 in range(4):
                                nc.tensor.matmul(
                                    ps[:],
                                    kT8[:, i2, :, ts(k, 128)],
                                    qT8[p][:, i2, :, CH : 2 * CH],
                                    start=(i2 == 0), stop=(i2 == 3),
                                    perf_mode=DR,
                                )
                            praw = p2.tile([128, CH], F16, name="praw", tag="praw", bufs=2)
                            nc.scalar.activation(
                                praw[:], ps[:], AF.Exp, scale=1.0 / 32.0
                            )
                            msk = p2.tile([128, CH], F16, name="msk", tag="msk", bufs=2)
                            nc.vector.tensor_scalar(
                                out=msk[:],
                                in0=qposB[:, ts(2 * p + 1, CH)],
                                scalar1=iota_sb[:, k : k + 1],
                                scalar2=None,
                                op0=ALU.is_ge,
                            )
                            nc.vector.tensor_tensor(
                                out=pt_ex[p][j][:], in0=praw[:], in1=msk[:],
                                op=ALU.mult,
                            )
                        # denominators per slot (FD=256 chains), then recip
                        for h_ in range(2):
                            sl = 2 * p + h_
                            dn_ps = pspool.tile([1, CH], F32, name="small", tag="at0", bufs=1)
                            nmm = LOHI[p][0] if h_ == 0 else hi
                            kk = 0
                            for k in range(lo):
                                nc.tensor.matmul(
                                    dn_ps[:], ones_col_bf[:],
                                    pt_sh[p][k][:, ts(h_, CH)],
                                    start=(kk == 0), stop=(kk == nmm - 1),
                                )
                                kk += 1
                                if h_ == 0 and kk == nmm:
                                    break
                            if h_ == 1:
                                for j in range(hi - lo):
                                    nc.tensor.matmul(
                                        dn_ps[:], ones_col_bf[:], pt_ex[p][j][:],
                                        start=(kk == 0), stop=(kk == nmm - 1),
                                    )
                                    kk += 1
                            dn_r = p2.tile([1, CH], F32R, name="dn_r", tag="dn_r", bufs=2)
                            nc.vector.tensor_copy(dn_r[:], dn_ps[:])
                            rb_ps = pspool.tile([128, CH], F32, name="small2", tag="at1", bufs=1)
                            nc.tensor.matmul(
                                rb_ps[:], ones_row[:], dn_r[:], start=True, stop=True
                            )
                            nc.vector.reciprocal(recipB[sl][:], rb_ps[:])

                    # pass B: attn^T = (P @ V)^T scaled by 1/denom
                    for p in range(2):
                        lo, hi = LOHI[p]
                        for half in range(2):
                            for d4 in range(4):
                                d_ = half * 4 + d4
                                aps = pspool.tile([128, 512], F32, name=f"at{d4}", tag=f"at{d4}", bufs=1)
                                for k in range(lo):
                                    nc.tensor.matmul(
                                        aps[:],
                                        vt[:, k, ts(d_, 128)],
                                        pt_sh[p][k][:],
                                        start=(k == 0), stop=False,
                                        skip_group_check=True,
                                    )
                                for j, k in enumerate(range(lo, hi)):
                                    nc.tensor.matmul(
                                        aps[:, CH : 2 * CH],
                                        vt[:, k, ts(d_, 128)],
                                        pt_ex[p][j][:],
                                        start=False, stop=(k == hi - 1),
                                        skip_group_check=True,
                                    )
                                if hi == lo:
                                    pass
                                for h_ in range(2):
                                    nc.vector.tensor_tensor(
                                        out=attnT[d_][p][:, ts(h_, CH)],
                                        in0=aps[:, ts(h_, CH)],
                                        in1=recipB[2 * p + h_][:],
                                        op=ALU.mult,
                                    )

                # ---------------- P3 + P4 ----------------
                with tc.tile_pool(name="p34", bufs=1) as p34:
                    outT = [
                        [p34.tile([128, 512], F16, name=f"oT{dt_}_{qb}", tag=f"oT{dt_}_{qb}") for qb in range(2)]
                        for dt_ in range(NDT)
                    ]
                    # P3: qb pair interleaved under one Wo weight block
                    for ot in range(NDT):
                        pso = [
                            pspool.tile([128, 512], F32, name=f"mmo{qb}", tag="mm", bufs=4)
                            for qb in range(2)
                        ]
                        for i in range(NDT):
                            for qb in range(2):
                                nc.tensor.matmul(
                                    pso[qb][:],
                                    wo_sb[:, i, ts(ot, 128)],
                                    attnT[i][qb][:],
                                    start=(i == 0), stop=(i == NDT - 1),
                                )
                        for qb in range(2):
                            nc.scalar.activation(
                                outT[ot][qb][:], pso[qb][:], AF.Identity,
                                bias=bo2_sb[:, ot : ot + 1],
                            )

                    # P4: FFN + GELU. Stationary = outT token-blocks, moving
                    # = the full 512-wide wf block: LDWEIGHTS hides under the
                    # previous matmul (K-pass pattern, 8-bank rotation). The
                    # free-dim bias bf is pre-filled into PSUM by the vector
                    # engine; matmuls accumulate on top (start=False).
                    bfbc = p34.tile([128, 8, 512], F16, name="bfbc", tag="bfbc")
                    for mb in range(M // 512):
                        bc_ps = pspool.tile([128, 512], F32, name="small", tag="at0", bufs=1)
                        nc.tensor.matmul(
                            bc_ps[:], ones_row[:], bfr_sb[:, ts(mb, 512)],
                            start=True, stop=True,
                        )
                        nc.scalar.activation(bfbc[:, mb, :], bc_ps[:], AF.Identity)
                    for mb in range(M // 512):
                        wfb = p34.tile([128, NDT, 512], F16, name="wfb", tag="wfb", bufs=2)
                        nc.gpsimd.dma_start(wfb[:], wfT[mb])
                        st = p34.tile([128, 8, 512], F16, name="ffstage", tag="ffstage", bufs=2)
                        ps8 = [
                            pspool.tile(
                                [128, 512], F32, name=f"ps8_{t8}",
                                tag=(f"at{t8}" if t8 < 4 else "mm"),
                                bufs=(1 if t8 < 4 else 4),
                            )
                            for t8 in range(8)
                        ]
                        for t8 in range(8):
                            nc.vector.tensor_copy(ps8[t8][:], bfbc[:, mb, :])
                        for i in range(NDT):
                            for t8 in range(8):
                                qb, tb2 = divmod(t8, 4)
                                nc.tensor.matmul(
                                    ps8[t8][:],
                                    outT[i][qb][:, ts(tb2, 128)],
                                    wfb[:, i, :],
                                    start=False, stop=(i == NDT - 1),
                                    skip_group_check=True,
                                )
                        for t8 in range(8):
                            nc.scalar.activation(st[:, t8, :], ps8[t8][:], AF.Gelu)
                            if t8 == 3:
                                nc.sync.dma_start(ffT[mb, :, 0:4, :], st[:, 0:4, :])
                        nc.sync.dma_start(ffT[mb, :, 4:8, :], st[:, 4:8, :])

    nc.compile()
    return nc


def _get_program():
    global _PROGRAM
    if _PROGRAM is None:
        _PROGRAM = _build_program()
    return _PROGRAM


def _owned_chunks(core):
    """The four 256-token chunk indices this core owns, in slot order."""
    if core % 2 == 0:
        return (0, 3, 4, 7)
    return (1, 2, 5, 6)


def _blocked(a):
    """[1024, W] -> [128, 8, W] with [p, i, c] = a[i*128+p, c]."""
    W = a.shape[1]
    return np.ascontiguousarray(a.reshape(8, 128, W).transpose(1, 0, 2))


def _make_in_maps(x, Wq, bq, Wk, bk, Wv, bv, Wo, bo, Wf, bf):
    f32, f16 = np.float32, np.float16
    wqT = _blocked(np.asarray(Wq.T, dtype=f16))
    wkT = _blocked(np.asarray(Wk.T, dtype=f16))
    wvT = _blocked(np.asarray(Wv.T, dtype=f16))
    woT = _blocked(np.asarray(Wo.T, dtype=f16))
    # wfT[mb, p, i, c] = Wf.T[i*128+p, mb*512+c]
    wfT = np.ascontiguousarray(
        np.asarray(Wf.T, dtype=f16).reshape(8, 128, 8, 512).transpose(2, 1, 0, 3)
    )
    bo2 = (Wo.astype(np.float64) @ bv.astype(np.float64) + bo.astype(np.float64))
    bo2 = np.ascontiguousarray(bo2.astype(f32).reshape(D // 128, 128).T)
    bfT = np.ascontiguousarray(bf.reshape(M // 128, 128).T, dtype=f32)
    iota = (
        np.arange(128, dtype=f32)[:, None]
        + 128.0 * np.arange(S // 128, dtype=f32)[None, :]
    )
    shared = {
        "wqT": wqT, "wkT": wkT, "wvT": wvT, "woT": woT, "wfT": wfT,
        "bq": np.ascontiguousarray(bq.reshape(D // 128, 128).T, dtype=f32),
        "bk": np.ascontiguousarray(bk.reshape(D // 128, 128).T, dtype=f32),
        "bo2": bo2,
        "bfT": bfT,
        "bf_row": np.ascontiguousarray(bf[None, :].astype(f32)),
        "iota_kt": np.ascontiguousarray(iota),
    }
    in_maps = []
    for core in range(N_CORES):
        b = core // 2
        chunks = _owned_chunks(core)
        xTb = np.asarray(x[b].T, dtype=f16)  # [D, S]
        half = core % 2  # rank within the pair: rank0 owns tokens 0:S/2
        xaT = _blocked(xTb[:, half * (S // 2) : (half + 1) * (S // 2)])
        xqT = _blocked(
            np.concatenate([xTb[:, c * CH : (c + 1) * CH] for c in chunks], axis=1)
        )
        qp = np.concatenate(
            [np.arange(c * CH, (c + 1) * CH) for c in chunks]
        ).astype(f32)[None, :]
        in_maps.append(
            {**shared, "xaT": xaT, "xqT": xqT,
             "qpos": np.ascontiguousarray(qp)}
        )
    return in_maps


def _run(inputs, trace=False, trace_cores=None, tmpdir=None):
    import sys

    if "/opt/trn_rl_repo" not in sys.path:
        sys.path.insert(0, "/opt/trn_rl_repo")
    from concourse.bass_utils import run_bass_kernel_spmd

    nc = _get_program()
    in_maps = _make_in_maps(**inputs)
    res = run_bass_kernel_spmd(
        nc, in_maps, list(range(N_CORES)), trace=trace, trace_cores=trace_cores,
        tmpdir=tmpdir,
    )
    out = np.empty((B, S, M), dtype=np.float32)
    for core in range(N_CORES):
        b = core // 2
        chunks = _owned_chunks(core)
        # ffT[mb, p, t8, c] = ff[(t8//4)*512 + (t8%4)*128 + p, mb*512 + c]
        raw = res.results[core]["ffT"].reshape(8, 128, 2, 4, 512)
        ff = np.ascontiguousarray(
            raw.transpose(2, 3, 1, 0, 4)
        ).reshape(4 * CH, M)
        for sl, c in enumerate(chunks):
            qb, qo = divmod(sl, 2)
            out[b, c * CH : (c + 1) * CH] = (
                ff[qb * 512 + qo * CH : qb * 512 + (qo + 1) * CH].astype(np.float32)
            )
    return out, res


def kernel(**inputs):
    out, _ = _run(inputs)
    return out


# revision 20
# speedup vs baseline: 1.2393x; 1.0144x over previous
"""Decoder block (single-head causal attention + GELU FFN) on 8 TRN2 NeuronCores.

Sharding: data parallel over batch (2 cores per batch), with the K AND V
projections token-split across the pair (each core projects its own half of
the sequence, then a pairwise AllGather shares both). Core c handles batch
b = c//2 and 1024 query tokens of that batch, chosen as four 256-token chunks
that balance the causal-attention workload:
  even cores (half 0): chunks 0, 3, 4, 7
  odd  cores (half 1): chunks 1, 2, 5, 6
The slot pairing makes the static k-tile counts per slot (4, 8, 12, 16) cover
both cores' needs with minimal waste (ideal is 36 tiles vs 40; the gap is
zeroed by the data-driven qpos mask). The SPMD program is identical on every
core; all per-core differences are data.

Performance structure (v2):
  - every matmul operand is fp16 (same PE rate as fp32r, half the DMA/SBUF)
  - V projection runs over the core's OWN half only (was: full sequence);
    K and V are exchanged by two pairwise AllGathers (K fires early, V later;
    P2 is restructured into scores-first/PV-second passes so the V gather
    latency hides behind all-slot score computation)
  - all weights / x tiles are multi-dim SBUF tiles filled by ONE or TWO big
    DMAs each (descriptor-generation on the issuing engine was costing
    ~630ns per 128KB tile; big transfers cut the issue count ~7x)
  - host pre-arranges every DRAM operand so each big DMA is contiguous per
    partition line (16KB runs)
  - Q/Wo/FFN matmuls interleave the two 512-token column blocks under one
    stationary weight load (halves LDWEIGHTS pressure; the pair partner's
    load hides under the 213ns FD=512 matmul)
  - P4 stages GELU results per 512-row block and stores once per block from
    the sync engine (64 -> 8 stores)
  - scalar engine runs only Identity in P1 and only Exp in P2 (activation
    table reloads cost 1.3us each)
"""

import numpy as np

D = 1024  # model dim
S = 2048  # sequence length
B = 4  # batch
M = 4096  # FFN dim
CH = 256  # q chunk (slot) size
NSLOT = 4  # q slots per core
NDT = D // 128  # 8 d-tiles
N_CORES = 8
NKT = [4, 8, 12, 16]  # k-tiles per slot (static max over the two paired cores)

_PROGRAM = None  # cached compiled program


def _build_program():
    import sys

    if "/opt/trn_rl_repo" not in sys.path:
        sys.path.insert(0, "/opt/trn_rl_repo")
    import concourse.bass as bass
    import concourse.tile as tile
    import concourse.mybir as mybir
    from concourse import bacc
    from concourse.bass import ts

    dt = mybir.dt
    AF = mybir.ActivationFunctionType
    ALU = mybir.AluOpType
    F32, F32R, F16, F8 = dt.float32, dt.float32r, dt.float16, dt.float8e4
    DR = mybir.MatmulPerfMode.DoubleRow

    nc = bacc.Bacc("TRN2", target_bir_lowering=False, debug=False, num_devices=8)

    # ---------------- DRAM I/O (all host-pre-arranged layouts) ----------------
    # weights: [128, i(8), 1024] with [p, i, c] = W.T[i*128+p, c]
    wqT = nc.dram_tensor("wqT", [128, NDT, D], F16, kind="ExternalInput").ap()
    wkT = nc.dram_tensor("wkT", [128, NDT, D], F16, kind="ExternalInput").ap()
    wvT = nc.dram_tensor("wvT", [128, NDT, D], F16, kind="ExternalInput").ap()
    woT = nc.dram_tensor("woT", [128, NDT, D], F16, kind="ExternalInput").ap()
    # FFN weight: [mb(8), 128, i(8), 512] with [mb, p, i, c] = Wf.T[i*128+p, mb*512+c]
    wfT = nc.dram_tensor("wfT", [M // 512, 128, NDT, 512], F16, kind="ExternalInput").ap()
    # x, own-half tokens in k order: [p, i, t] = x.T[i*128+p, half*1024+t]
    xaT = nc.dram_tensor("xaT", [128, NDT, S // 2], F16, kind="ExternalInput").ap()
    # x, own 4 chunks in q order
    xqT = nc.dram_tensor("xqT", [128, NDT, 4 * CH], F16, kind="ExternalInput").ap()
    bq = nc.dram_tensor("bq", [128, D // 128], F32, kind="ExternalInput").ap()
    bk = nc.dram_tensor("bk", [128, D // 128], F32, kind="ExternalInput").ap()
    bo2 = nc.dram_tensor("bo2", [128, D // 128], F32, kind="ExternalInput").ap()
    bfT = nc.dram_tensor("bfT", [128, M // 128], F32, kind="ExternalInput").ap()
    qpos = nc.dram_tensor("qpos", [1, 4 * CH], F32R, kind="ExternalInput").ap()
    bf_row = nc.dram_tensor("bf_row", [1, M], F32R, kind="ExternalInput").ap()
    iota_kt = nc.dram_tensor("iota_kt", [128, S // 128], F32, kind="ExternalInput").ap()
    # output: [mb(8), 128p, t8(8), 512c] = ff[(t8//4)*512+(t8%4)*128+p, mb*512+c]
    ffT = nc.dram_tensor("ffT", [M // 512, 128, 8, 512], F16, kind="ExternalOutput").ap()

    with tile.TileContext(nc) as tc:
        with (
            tc.tile_pool(name="const", bufs=1) as cpool,
            tc.tile_pool(name="psum", bufs=1, space="PSUM") as pspool,
        ):
            # ---------------- constants (scalar engine issues these) --------
            ones_col_bf = cpool.tile([128, 1], F16, name="ones_col_bf", tag="ones_col_bf")
            nc.vector.memset(ones_col_bf[:], 1.0)
            ones_row_f = cpool.tile([1, 128], F32, name="ones_row_f", tag="ones_row_f")
            nc.vector.memset(ones_row_f[:], 1.0)
            ones_row = cpool.tile([1, 128], F32R, name="ones_row", tag="ones_row")
            nc.vector.tensor_copy(ones_row[:], ones_row_f[:])
            iota_sb = cpool.tile([128, S // 128], F32, name="iota", tag="iota")
            nc.scalar.dma_start(iota_sb[:], iota_kt[:])
            bq_sb = cpool.tile([128, D // 128], F32, name="bq", tag="bq")
            nc.scalar.dma_start(bq_sb[:], bq[:])
            bk_sb = cpool.tile([128, D // 128], F32, name="bk", tag="bk")
            nc.scalar.dma_start(bk_sb[:], bk[:])
            bo2_sb = cpool.tile([128, D // 128], F32, name="bo2", tag="bo2")
            nc.scalar.dma_start(bo2_sb[:], bo2[:])
            bf_sb = cpool.tile([128, M // 128], F32, name="bf", tag="bf")
            nc.scalar.dma_start(bf_sb[:], bfT[:])
            qpos_row = cpool.tile([1, 4 * CH], F32R, name="qpos_row", tag="qpos_row")
            nc.scalar.dma_start(qpos_row[:], qpos[:])
            bfr_sb = cpool.tile([1, M], F32R, name="bfr_sb", tag="bfr_sb")
            nc.scalar.dma_start(bfr_sb[:], bf_row[:])
            qposB = cpool.tile([128, 4 * CH], F32, name="qposB", tag="qposB")

            # ------------- long-lived tiles: one pool spanning P1..P4 -------
            with (
                tc.tile_pool(name="main", bufs=1) as mp,
                tc.tile_pool(name="dram", bufs=1, space="DRAM") as dram,
            ):
                # fp8 K^T in DoubleRow pair layout: [p, i2, s, tok],
                # contraction d = (2*i2+s)*128 + p
                kT8 = mp.tile([128, 4, 2, S], F8, name="kT8", tag="kT8")
                vt = mp.tile([128, 16, D], F16, name="vt", tag="vt")
                wq_sb = mp.tile([128, NDT, D], F16, name="wq", tag="wq")
                wo_sb = mp.tile([128, NDT, D], F16, name="wo", tag="wo")
                xq = mp.tile([128, NDT, 4 * CH], F16, name="xq", tag="xq")
                qT8 = [
                    mp.tile([128, 4, 2, 512], F8, name=f"qT8_{qb}", tag=f"qT8_{qb}")
                    for qb in range(2)
                ]
                attnT = [
                    [mp.tile([128, 512], F16, name=f"at{dt_}_{qb}", tag=f"at{dt_}_{qb}") for qb in range(2)]
                    for dt_ in range(NDT)
                ]
                # DRAM bounce buffers for the pairwise K and V AllGathers.
                # Each projection is gathered in two 1MB halves so the
                # collectives fire earlier and finish well before P2 needs
                # the peer's tokens.
                ka_in = dram.tile([128, 4, 2, 512], F8, name="ka_in", tag="ka_in")
                ka_out = dram.tile([2, 128, 4, 2, 512], F8, name="ka_out", tag="ka_out")
                kb_in = dram.tile([128, 4, 2, 512], F8, name="kb_in", tag="kb_in")
                kb_out = dram.tile([2, 128, 4, 2, 512], F8, name="kb_out", tag="kb_out")
                va_in = dram.tile([128, 4, D], F16, name="va_in", tag="va_in")
                va_out = dram.tile([2, 128, 4, D], F16, name="va_out", tag="va_out")
                vb_in = dram.tile([128, 4, D], F16, name="vb_in", tag="vb_in")
                vb_out = dram.tile([2, 128, 4, D], F16, name="vb_out", tag="vb_out")

                def pair_gather(in_t, out_t):
                    nc.gpsimd.collective_compute(
                        "AllGather",
                        mybir.AluOpType.bypass,
                        replica_groups=[[0, 1], [2, 3], [4, 5], [6, 7]],
                        ins=[in_t[:].opt()],
                        outs=[out_t[:].opt()],
                    )

                # ---------------- P1 ----------------
                with tc.tile_pool(name="p1a", bufs=1) as p1a:
                    wk_sb = p1a.tile([128, NDT, D], F16, name="wk", tag="wk")
                    wv_sb = p1a.tile([128, NDT, D], F16, name="wv", tag="wv")
                    xa = p1a.tile([128, NDT, S // 2], F16, name="xa", tag="xa")

                    # DMA issue is tiered: all in-flight DMAs share wire
                    # bandwidth, so only the critical-path wk/xa stream starts
                    # immediately (per-i granularity: compute starts on the
                    # first 256KB). Later streams are gated on compute
                    # sentinels and issued from the otherwise-idle vector
                    # engine so they cannot steal bandwidth early.
                    for h in range(2):
                        for i in range(NDT):
                            nc.gpsimd.dma_start(
                                wk_sb[:, i, ts(h, 512)], wkT[:, i, ts(h, 512)]
                            )
                            nc.sync.dma_start(
                                xa[:, i, ts(h, 512)], xaT[:, i, ts(h, 512)]
                            )

                    # ---- K projection over own half (kT staged at [:, :, 0:1024])
                    for tb in range(2):
                        for og in range(2):
                            ps4 = [
                                pspool.tile([128, 512], F32, name=f"at{j}", tag=f"at{j}", bufs=1)
                                for j in range(4)
                            ]
                            for i in range(NDT):
                                for j in range(4):
                                    nc.tensor.matmul(
                                        ps4[j][:],
                                        wk_sb[:, i, ts(og * 4 + j, 128)],
                                        xa[:, i, ts(tb, 512)],
                                        start=(i == 0), stop=(i == NDT - 1),
                                    )
                            for j in range(4):
                                ot = og * 4 + j
                                nc.scalar.activation(
                                    kT8[:, ot // 2, ot % 2, ts(tb, 512)], ps4[j][:],
                                    AF.Identity, bias=bk_sb[:, ot : ot + 1],
                                )
                            if tb == 0 and og == 0:
                                # tier-1 issue: scalar reaches here only after
                                # og0's drains, so wv/xq cannot steal wire
                                # bandwidth from the critical wk/xa stream
                                for h in range(2):
                                    nc.scalar.dma_start(
                                        wv_sb[:, ts(h, 4), :], wvT[:, ts(h, 4), :]
                                    )
                                nc.scalar.dma_start(xq[:], xqT[:])
                        # stage this token block and fire its K gather
                        kin = ka_in if tb == 0 else kb_in
                        nc.sync.dma_start(kin[:], kT8[:, :, :, ts(tb, 512)])
                        pair_gather(kin, ka_out if tb == 0 else kb_out)

                    # ---- V projection over own half (token-major, 2 banks/tt)
                    for tb in range(2):
                        for tt in range(4):
                            ps2 = [
                                pspool.tile([128, 512], F32, name=f"mm{ob}", tag="mm", bufs=4)
                                for ob in range(2)
                            ]
                            for i in range(NDT):
                                for ob in range(2):
                                    nc.tensor.matmul(
                                        ps2[ob][:],
                                        xa[:, i, tb * 512 + tt * 128 : tb * 512 + (tt + 1) * 128],
                                        wv_sb[:, i, ts(ob, 512)],
                                        start=(i == 0), stop=(i == NDT - 1),
                                    )
                            for ob in range(2):
                                nc.scalar.activation(
                                    vt[:, tb * 4 + tt, ts(ob, 512)], ps2[ob][:], AF.Identity
                                )
                            if tb == 0 and tt == 0:
                                # tier-2 issue (see tier-1 note)
                                for h in range(2):
                                    nc.scalar.dma_start(
                                        wq_sb[:, ts(h, 4), :], wqT[:, ts(h, 4), :]
                                    )
                        # stage this V token block and fire its gather.
                        # vt[:, 0:8] doubles as the projection scratch; the
                        # readback overwrites all 16 slots in token order.
                        vin = va_in if tb == 0 else vb_in
                        nc.scalar.dma_start(vin[:], vt[:, ts(tb, 4), :])
                        pair_gather(vin, va_out if tb == 0 else vb_out)

                    # qpos broadcast (fills the gather window)
                    for i in range(4 * CH // 512):
                        bc_ps = pspool.tile([128, 512], F32, name="small", tag="at0", bufs=1)
                        nc.tensor.matmul(
                            bc_ps[:], ones_row[:], qpos_row[:, ts(i, 512)],
                            start=True, stop=True,
                        )
                        nc.scalar.activation(qposB[:, ts(i, 512)], bc_ps[:], AF.Identity)

                    # ---- Q projection: qb pair interleaved under one weight
                    for ot in range(NDT):
                        psq = [
                            pspool.tile([128, 512], F32, name=f"mmq{qb}", tag="mm", bufs=4)
                            for qb in range(2)
                        ]
                        for i in range(NDT):
                            for qb in range(2):
                                nc.tensor.matmul(
                                    psq[qb][:],
                                    wq_sb[:, i, ts(ot, 128)],
                                    xq[:, i, ts(qb, 512)],
                                    start=(i == 0), stop=(i == NDT - 1),
                                )
                        for qb in range(2):
                            nc.scalar.activation(
                                qT8[qb][:, ot // 2, ot % 2, :], psq[qb][:],
                                AF.Identity, bias=bq_sb[:, ot : ot + 1],
                            )
                        if ot == 0:
                            # tier-3: wo is only needed in P3; issuing it this
                            # late keeps it off the K/V gather's wire window
                            for h in range(2):
                                nc.scalar.dma_start(
                                    wo_sb[:, ts(h, 4), :], woT[:, ts(h, 4), :]
                                )

                # gathered K^T / V readback in true token order, ordered by
                # first use in P2 (sync engine; waits ride on the collectives)
                for r in range(2):
                    nc.sync.dma_start(
                        kT8[:, :, :, r * 1024 + 0 : r * 1024 + 512], ka_out[r]
                    )
                    nc.sync.dma_start(
                        kT8[:, :, :, r * 1024 + 512 : r * 1024 + 1024], kb_out[r]
                    )
                for r in range(2):
                    nc.sync.dma_start(vt[:, r * 8 + 0 : r * 8 + 4, :], va_out[r])
                    nc.sync.dma_start(vt[:, r * 8 + 4 : r * 8 + 8, :], vb_out[r])

                # ---------------- P2: attention ----------------
                # Slots are processed as PAIRS sharing a 512-token q block:
                # the k-range both slots need runs at FD=512 (LDWEIGHTS fully
                # hidden), the hi-slot's excess k-tiles at FD=256.
                # pass A computes scores+exp+mask+denom for all slots first
                # (probs stay resident), so the V gather hides behind it.
                with tc.tile_pool(name="p2", bufs=1) as p2:
                    LOHI = [(NKT[0], NKT[1]), (NKT[2], NKT[3])]
                    pt_sh = [
                        [
                            p2.tile([128, 512], F16, name=f"pts{p}_{k}", tag=f"pts{p}_{k}", bufs=1)
                            for k in range(LOHI[p][0])
                        ]
                        for p in range(2)
                    ]
                    pt_ex = [
                        [
                            p2.tile([128, CH], F16, name=f"ptx{p}_{j}", tag=f"ptx{p}_{j}", bufs=1)
                            for j in range(LOHI[p][1] - LOHI[p][0])
                        ]
                        for p in range(2)
                    ]
                    recipB = [
                        p2.tile([128, CH], F32, name=f"recipB{sl}", tag=f"recipB{sl}", bufs=1)
                        for sl in range(NSLOT)
                    ]
                    for p in range(2):
                        lo, hi = LOHI[p]
                        # shared k-range: FD=512 over both slots
                        for k in range(lo):
                            ps = pspool.tile([128, 512], F32, name="mm", tag="mm", bufs=4)
                            for i2 in range(4):
                                nc.tensor.matmul(
                                    ps[:],
                                    kT8[:, i2, :, ts(k, 128)],
                                    qT8[p][:, i2, :, :],
                                    start=(i2 == 0), stop=(i2 == 3),
                                    perf_mode=DR,
                                )
                            nc.scalar.activation(
                                pt_sh[p][k][:], ps[:], AF.Exp, scale=1.0 / 32.0
                            )
                            if k >= lo - 4:
                                # mask applies to the lo slot's columns only
                                msk = p2.tile([128, CH], F16, name="msk", tag="msk", bufs=2)
                                nc.vector.tensor_scalar(
                                    out=msk[:],
                                    in0=qposB[:, ts(2 * p, CH)],
                                    scalar1=iota_sb[:, k : k + 1],
                                    scalar2=None,
                                    op0=ALU.is_ge,
                                )
                                nc.vector.tensor_tensor(
                                    out=pt_sh[p][k][:, 0:CH],
                                    in0=pt_sh[p][k][:, 0:CH], in1=msk[:],
                                    op=ALU.mult,
                                )
                        # excess k-tiles: hi slot only, FD=256, all masked
                        for j, k in enumerate(range(lo, hi)):
                            ps = pspool.tile([128, CH], F32, name="mm", tag="mm", bufs=4)
                            for i2 in range(4):
                                nc.tensor.matmul(
                                    ps[:],
                                    kT8[:, i2, :, ts(k, 128)],
                                    qT8[p][:, i2, :, CH : 2 * CH],
                                    start=(i2 == 0), stop=(i2 == 3),
                                    perf_mode=DR,
                                )
                            praw = p2.tile([128, CH], F16, name="praw", tag="praw", bufs=2)
                            nc.scalar.activation(
                                praw[:], ps[:], AF.Exp, scale=1.0 / 32.0
                            )
                            msk = p2.tile([128, CH], F16, name="msk", tag="msk", bufs=2)
                            nc.vector.tensor_scalar(
                                out=msk[:],
                                in0=qposB[:, ts(2 * p + 1, CH)],
                                scalar1=iota_sb[:, k : k + 1],
                                scalar2=None,
                                op0=ALU.is_ge,
                            )
                            nc.vector.tensor_tensor(
                                out=pt_ex[p][j][:], in0=praw[:], in1=msk[:],
                                op=ALU.mult,
                            )
                        # denominators per slot (FD=256 chains), then recip
                        for h_ in range(2):
                            sl = 2 * p + h_
                            dn_ps = pspool.tile([1, CH], F32, name="small", tag="at0", bufs=1)
                            nmm = LOHI[p][0] if h_ == 0 else hi
                            kk = 0
                            for k in range(lo):
                                nc.tensor.matmul(
                                    dn_ps[:], ones_col_bf[:],
                                    pt_sh[p][k][:, ts(h_, CH)],
                                    start=(kk == 0), stop=(kk == nmm - 1),
                                )
                                kk += 1
                                if h_ == 0 and kk == nmm:
                                    break
                            if h_ == 1:
                                for j in range(hi - lo):
                                    nc.tensor.matmul(
                                        dn_ps[:], ones_col_bf[:], pt_ex[p][j][:],
                                        start=(kk == 0), stop=(kk == nmm - 1),
                                    )
                                    kk += 1
                            dn_r = p2.tile([1, CH], F32R, name="dn_r", tag="dn_r", bufs=2)
                            nc.vector.tensor_copy(dn_r[:], dn_ps[:])
                            rb_ps = pspool.tile([128, CH], F32, name="small2", tag="at1", bufs=1)
                            nc.tensor.matmul(
                                rb_ps[:], ones_row[:], dn_r[:], start=True, stop=True
                            )
                            nc.vector.reciprocal(recipB[sl][:], rb_ps[:])

                    # pass B: attn^T = (P @ V)^T scaled by 1/denom
                    for p in range(2):
                        lo, hi = LOHI[p]
                        for half in range(2):
                            for d4 in range(4):
                                d_ = half * 4 + d4
                                aps = pspool.tile([128, 512], F32, name=f"at{d4}", tag=f"at{d4}", bufs=1)
                                for k in range(lo):
                                    nc.tensor.matmul(
                                        aps[:],
                                        vt[:, k, ts(d_, 128)],
                                        pt_sh[p][k][:],
                                        start=(k == 0), stop=False,
                                        skip_group_check=True,
                                    )
                                for j, k in enumerate(range(lo, hi)):
                                    nc.tensor.matmul(
                                        aps[:, CH : 2 * CH],
                                        vt[:, k, ts(d_, 128)],
                                        pt_ex[p][j][:],
                                        start=False, stop=(k == hi - 1),
                                        skip_group_check=True,
                                    )
                                if hi == lo:
                                    pass
                                for h_ in range(2):
                                    nc.vector.tensor_tensor(
                                        out=attnT[d_][p][:, ts(h_, CH)],
                                        in0=aps[:, ts(h_, CH)],
                                        in1=recipB[2 * p + h_][:],
                                        op=ALU.mult,
                                    )

                # ---------------- P3 + P4 ----------------
                with tc.tile_pool(name="p34", bufs=1) as p34:
                    outT = [
                        [p34.tile([128, 512], F16, name=f"oT{dt_}_{qb}", tag=f"oT{dt_}_{qb}") for qb in range(2)]
                        for dt_ in range(NDT)
                    ]
                    # P3: qb pair interleaved under one Wo weight block
                    for ot in range(NDT):
                        pso = [
                            pspool.tile([128, 512], F32, name=f"mmo{qb}", tag="mm", bufs=4)
                            for qb in range(2)
                        ]
                        for i in range(NDT):
                            for qb in range(2):
                                nc.tensor.matmul(
                                    pso[qb][:],
                                    wo_sb[:, i, ts(ot, 128)],
                                    attnT[i][qb][:],
                                    start=(i == 0), stop=(i == NDT - 1),
                                )
                        for qb in range(2):
                            nc.scalar.activation(
                                outT[ot][qb][:], pso[qb][:], AF.Identity,
                                bias=bo2_sb[:, ot : ot + 1],
                            )

                    # P4: FFN + GELU. Stationary = outT token-blocks, moving
                    # = the full 512-wide wf block: LDWEIGHTS hides under the
                    # previous matmul (K-pass pattern, 8-bank rotation). The
                    # free-dim bias bf is pre-filled into PSUM by the vector
                    # engine; matmuls accumulate on top (start=False).
                    bfbc = p34.tile([128, 8, 512], F16, name="bfbc", tag="bfbc")
                    for mb in range(M // 512):
                        bc_ps = pspool.tile([128, 512], F32, name="small", tag="at0", bufs=1)
                        nc.tensor.matmul(
                            bc_ps[:], ones_row[:], bfr_sb[:, ts(mb, 512)],
                            start=True, stop=True,
                        )
                        nc.scalar.activation(bfbc[:, mb, :], bc_ps[:], AF.Identity)
                    for mb in range(M // 512):
                        wfb = p34.tile([128, NDT, 512], F16, name="wfb", tag="wfb", bufs=2)
                        nc.gpsimd.dma_start(wfb[:], wfT[mb])
                        st = p34.tile([128, 8, 512], F16, name="ffstage", tag="ffstage", bufs=2)
                        ps8 = [
                            pspool.tile(
                                [128, 512], F32, name=f"ps8_{t8}",
                                tag=(f"at{t8}" if t8 < 4 else "mm"),
                                bufs=(1 if t8 < 4 else 4),
                            )
                            for t8 in range(8)
                        ]
                        for t8 in range(8):
                            qb, tb2 = divmod(t8, 4)
                            nc.vector.tensor_copy(ps8[t8][:], bfbc[:, mb, :])
                            for i in range(NDT):
                                nc.tensor.matmul(
                                    ps8[t8][:],
                                    outT[i][qb][:, ts(tb2, 128)],
                                    wfb[:, i, :],
                                    start=False, stop=(i == NDT - 1),
                                    skip_group_check=True,
                                )
                            nc.scalar.activation(st[:, t8, :], ps8[t8][:], AF.Gelu)
                            if t8 == 3:
                                nc.sync.dma_start(ffT[mb, :, 0:4, :], st[:, 0:4, :])
                        nc.sync.dma_start(ffT[mb, :, 4:8, :], st[:, 4:8, :])

    nc.compile()
    return nc


def _get_program():
    global _PROGRAM
    if _PROGRAM is None:
        _PROGRAM = _build_program()
    return _PROGRAM


def _owned_chunks(core):
    """The four 256-token chunk indices this core owns, in slot order."""
    if core % 2 == 0:
        return (0, 3, 4, 7)
    return (1, 2, 5, 6)


def _blocked(a):
    """[1024, W] -> [128, 8, W] with [p, i, c] = a[i*128+p, c]."""
    W = a.shape[1]
    return np.ascontiguousarray(a.reshape(8, 128, W).transpose(1, 0, 2))


def _make_in_maps(x, Wq, bq, Wk, bk, Wv, bv, Wo, bo, Wf, bf):
    f32, f16 = np.float32, np.float16
    wqT = _blocked(np.asarray(Wq.T, dtype=f16))
    wkT = _blocked(np.asarray(Wk.T, dtype=f16))
    wvT = _blocked(np.asarray(Wv.T, dtype=f16))
    woT = _blocked(np.asarray(Wo.T, dtype=f16))
    # wfT[mb, p, i, c] = Wf.T[i*128+p, mb*512+c]
    wfT = np.ascontiguousarray(
        np.asarray(Wf.T, dtype=f16).reshape(8, 128, 8, 512).transpose(2, 1, 0, 3)
    )
    bo2 = (Wo.astype(np.float64) @ bv.astype(np.float64) + bo.astype(np.float64))
    bo2 = np.ascontiguousarray(bo2.astype(f32).reshape(D // 128, 128).T)
    bfT = np.ascontiguousarray(bf.reshape(M // 128, 128).T, dtype=f32)
    iota = (
        np.arange(128, dtype=f32)[:, None]
        + 128.0 * np.arange(S // 128, dtype=f32)[None, :]
    )
    shared = {
        "wqT": wqT, "wkT": wkT, "wvT": wvT, "woT": woT, "wfT": wfT,
        "bq": np.ascontiguousarray(bq.reshape(D // 128, 128).T, dtype=f32),
        "bk": np.ascontiguousarray(bk.reshape(D // 128, 128).T, dtype=f32),
        "bo2": bo2,
        "bfT": bfT,
        "bf_row": np.ascontiguousarray(bf[None, :].astype(f32)),
        "iota_kt": np.ascontiguousarray(iota),
    }
    in_maps = []
    for core in range(N_CORES):
        b = core // 2
        chunks = _owned_chunks(core)
        xTb = np.asarray(x[b].T, dtype=f16)  # [D, S]
        half = core % 2  # rank within the pair: rank0 owns tokens 0:S/2
        xaT = _blocked(xTb[:, half * (S // 2) : (half + 1) * (S // 2)])
        xqT = _blocked(
            np.concatenate([xTb[:, c * CH : (c + 1) * CH] for c in chunks], axis=1)
        )
        qp = np.concatenate(
            [np.arange(c * CH, (c + 1) * CH) for c in chunks]
        ).astype(f32)[None, :]
        in_maps.append(
            {**shared, "xaT": xaT, "xqT": xqT,
             "qpos": np.ascontiguousarray(qp)}
        )
    return in_maps


def _run(inputs, trace=False, trace_cores=None, tmpdir=None):
    import sys

    if "/opt/trn_rl_repo" not in sys.path:
        sys.path.insert(0, "/opt/trn_rl_repo")
    from concourse.bass_utils import run_bass_kernel_spmd

    nc = _get_program()
    in_maps = _make_in_maps(**inputs)
    res = run_bass_kernel_spmd(
        nc, in_maps, list(range(N_CORES)), trace=trace, trace_cores=trace_cores,
        tmpdir=tmpdir,
    )
    out = np.empty((B, S, M), dtype=np.float32)
    for core in range(N_CORES):
        b = core // 2
        chunks = _owned_chunks(core)
        # ffT[mb, p, t8, c] = ff[(t8//4)*512 + (t8%4)*128 + p, mb*512 + c]
        raw = res.results[core]["ffT"].reshape(8, 128, 2, 4, 512)
        ff = np.ascontiguousarray(
            raw.transpose(2, 3, 1, 0, 4)
        ).reshape(4 * CH, M)
        for sl, c in enumerate(chunks):
            qb, qo = divmod(sl, 2)
            out[b, c * CH : (c + 1) * CH] = (
                ff[qb * 512 + qo * CH : qb * 512 + (qo + 1) * CH].astype(np.float32)
            )
    return out, res


def kernel(**inputs):
    out, _ = _run(inputs)
    return out


# revision 22
# speedup vs baseline: 1.2440x; 1.0038x over previous
"""Decoder block (single-head causal attention + GELU FFN) on 8 TRN2 NeuronCores.

Sharding: data parallel over batch (2 cores per batch), with the K AND V
projections token-split across the pair (each core projects its own half of
the sequence, then a pairwise AllGather shares both). Core c handles batch
b = c//2 and 1024 query tokens of that batch, chosen as four 256-token chunks
that balance the causal-attention workload:
  even cores (half 0): chunks 0, 3, 4, 7
  odd  cores (half 1): chunks 1, 2, 5, 6
The slot pairing makes the static k-tile counts per slot (4, 8, 12, 16) cover
both cores' needs with minimal waste (ideal is 36 tiles vs 40; the gap is
zeroed by the data-driven qpos mask). The SPMD program is identical on every
core; all per-core differences are data.

Performance structure (v2):
  - every matmul operand is fp16 (same PE rate as fp32r, half the DMA/SBUF)
  - V projection runs over the core's OWN half only (was: full sequence);
    K and V are exchanged by two pairwise AllGathers (K fires early, V later;
    P2 is restructured into scores-first/PV-second passes so the V gather
    latency hides behind all-slot score computation)
  - all weights / x tiles are multi-dim SBUF tiles filled by ONE or TWO big
    DMAs each (descriptor-generation on the issuing engine was costing
    ~630ns per 128KB tile; big transfers cut the issue count ~7x)
  - host pre-arranges every DRAM operand so each big DMA is contiguous per
    partition line (16KB runs)
  - Q/Wo/FFN matmuls interleave the two 512-token column blocks under one
    stationary weight load (halves LDWEIGHTS pressure; the pair partner's
    load hides under the 213ns FD=512 matmul)
  - P4 stages GELU results per 512-row block and stores once per block from
    the sync engine (64 -> 8 stores)
  - scalar engine runs only Identity in P1 and only Exp in P2 (activation
    table reloads cost 1.3us each)
"""

import numpy as np
import ml_dtypes

_E4M3 = ml_dtypes.float8_e4m3fn

D = 1024  # model dim
S = 2048  # sequence length
B = 4  # batch
M = 4096  # FFN dim
CH = 256  # q chunk (slot) size
NSLOT = 4  # q slots per core
NDT = D // 128  # 8 d-tiles
N_CORES = 8
NKT = [4, 8, 12, 16]  # k-tiles per slot (static max over the two paired cores)

_PROGRAM = None  # cached compiled program


def _build_program():
    import sys

    if "/opt/trn_rl_repo" not in sys.path:
        sys.path.insert(0, "/opt/trn_rl_repo")
    import concourse.bass as bass
    import concourse.tile as tile
    import concourse.mybir as mybir
    from concourse import bacc
    from concourse.bass import ts

    dt = mybir.dt
    AF = mybir.ActivationFunctionType
    ALU = mybir.AluOpType
    F32, F32R, F16, F8 = dt.float32, dt.float32r, dt.float16, dt.float8e4
    DR = mybir.MatmulPerfMode.DoubleRow

    nc = bacc.Bacc("TRN2", target_bir_lowering=False, debug=False, num_devices=8)

    # ---------------- DRAM I/O (all host-pre-arranged layouts) ----------------
    # fp16 weights: [128, i(8), 1024] with [p, i, c] = W.T[i*128+p, c]
    wvT = nc.dram_tensor("wvT", [128, NDT, D], F16, kind="ExternalInput").ap()
    woT = nc.dram_tensor("woT", [128, NDT, D], F16, kind="ExternalInput").ap()
    # fp8 QK weights in DoubleRow pair layout, host-scaled by 64:
    # [p, i2, s, c] = e4m3(64 * W.T[(2*i2+s)*128+p, c])
    wq8T = nc.dram_tensor("wq8T", [128, 4, 2, D], F8, kind="ExternalInput").ap()
    wk8T = nc.dram_tensor("wk8T", [128, 4, 2, D], F8, kind="ExternalInput").ap()
    # FFN weight: [mb(8), 128, i(8), 512] with [mb, p, i, c] = Wf.T[i*128+p, mb*512+c]
    wfT = nc.dram_tensor("wfT", [M // 512, 128, NDT, 512], F16, kind="ExternalInput").ap()
    # x, own-half tokens in k order: [p, i, t] = x.T[i*128+p, half*1024+t]
    xaT = nc.dram_tensor("xaT", [128, NDT, S // 2], F16, kind="ExternalInput").ap()
    # fp8 x copies in DoubleRow pair layout (for the Q/K projections)
    xa8T = nc.dram_tensor("xa8T", [128, 4, 2, S // 2], F8, kind="ExternalInput").ap()
    xq8T = nc.dram_tensor("xq8T", [128, 4, 2, 4 * CH], F8, kind="ExternalInput").ap()
    bq = nc.dram_tensor("bq", [128, D // 128], F32, kind="ExternalInput").ap()
    bk = nc.dram_tensor("bk", [128, D // 128], F32, kind="ExternalInput").ap()
    bo2 = nc.dram_tensor("bo2", [128, D // 128], F32, kind="ExternalInput").ap()
    bfT = nc.dram_tensor("bfT", [128, M // 128], F32, kind="ExternalInput").ap()
    qpos = nc.dram_tensor("qpos", [1, 4 * CH], F32R, kind="ExternalInput").ap()
    bf_row = nc.dram_tensor("bf_row", [1, M], F32R, kind="ExternalInput").ap()
    iota_kt = nc.dram_tensor("iota_kt", [128, S // 128], F32, kind="ExternalInput").ap()
    # output: [mb(8), 128p, t8(8), 512c] = ff[(t8//4)*512+(t8%4)*128+p, mb*512+c]
    ffT = nc.dram_tensor("ffT", [M // 512, 128, 8, 512], F16, kind="ExternalOutput").ap()

    with tile.TileContext(nc) as tc:
        with (
            tc.tile_pool(name="const", bufs=1) as cpool,
            tc.tile_pool(name="psum", bufs=1, space="PSUM") as pspool,
        ):
            # ---------------- constants (scalar engine issues these) --------
            ones_col_bf = cpool.tile([128, 1], F16, name="ones_col_bf", tag="ones_col_bf")
            nc.vector.memset(ones_col_bf[:], 1.0)
            ones_row_f = cpool.tile([1, 128], F32, name="ones_row_f", tag="ones_row_f")
            nc.vector.memset(ones_row_f[:], 1.0)
            ones_row = cpool.tile([1, 128], F32R, name="ones_row", tag="ones_row")
            nc.vector.tensor_copy(ones_row[:], ones_row_f[:])
            iota_sb = cpool.tile([128, S // 128], F32, name="iota", tag="iota")
            nc.scalar.dma_start(iota_sb[:], iota_kt[:])
            bq_sb = cpool.tile([128, D // 128], F32, name="bq", tag="bq")
            nc.scalar.dma_start(bq_sb[:], bq[:])
            bk_sb = cpool.tile([128, D // 128], F32, name="bk", tag="bk")
            nc.scalar.dma_start(bk_sb[:], bk[:])
            bo2_sb = cpool.tile([128, D // 128], F32, name="bo2", tag="bo2")
            nc.scalar.dma_start(bo2_sb[:], bo2[:])
            bf_sb = cpool.tile([128, M // 128], F32, name="bf", tag="bf")
            nc.scalar.dma_start(bf_sb[:], bfT[:])
            qpos_row = cpool.tile([1, 4 * CH], F32R, name="qpos_row", tag="qpos_row")
            nc.scalar.dma_start(qpos_row[:], qpos[:])
            bfr_sb = cpool.tile([1, M], F32R, name="bfr_sb", tag="bfr_sb")
            nc.scalar.dma_start(bfr_sb[:], bf_row[:])
            qposB = cpool.tile([128, 4 * CH], F32, name="qposB", tag="qposB")

            # ------------- long-lived tiles: one pool spanning P1..P4 -------
            with (
                tc.tile_pool(name="main", bufs=1) as mp,
                tc.tile_pool(name="dram", bufs=1, space="DRAM") as dram,
            ):
                # fp8 K^T in DoubleRow pair layout: [p, i2, s, tok],
                # contraction d = (2*i2+s)*128 + p
                kT8 = mp.tile([128, 4, 2, S], F8, name="kT8", tag="kT8")
                vt = mp.tile([128, 16, D], F16, name="vt", tag="vt")
                wo_sb = mp.tile([128, NDT, D], F16, name="wo", tag="wo")
                qT8 = [
                    mp.tile([128, 4, 2, 512], F8, name=f"qT8_{qb}", tag=f"qT8_{qb}")
                    for qb in range(2)
                ]
                attnT = [
                    [mp.tile([128, 512], F16, name=f"at{dt_}_{qb}", tag=f"at{dt_}_{qb}") for qb in range(2)]
                    for dt_ in range(NDT)
                ]
                # DRAM bounce buffers for the pairwise K and V AllGathers.
                # Each projection is gathered in two 1MB halves so the
                # collectives fire earlier and finish well before P2 needs
                # the peer's tokens.
                ka_in = dram.tile([128, 4, 2, 512], F8, name="ka_in", tag="ka_in")
                ka_out = dram.tile([2, 128, 4, 2, 512], F8, name="ka_out", tag="ka_out")
                kb_in = dram.tile([128, 4, 2, 512], F8, name="kb_in", tag="kb_in")
                kb_out = dram.tile([2, 128, 4, 2, 512], F8, name="kb_out", tag="kb_out")
                va_in = dram.tile([128, 4, D], F16, name="va_in", tag="va_in")
                va_out = dram.tile([2, 128, 4, D], F16, name="va_out", tag="va_out")
                vb_in = dram.tile([128, 4, D], F16, name="vb_in", tag="vb_in")
                vb_out = dram.tile([2, 128, 4, D], F16, name="vb_out", tag="vb_out")

                def pair_gather(in_t, out_t):
                    nc.gpsimd.collective_compute(
                        "AllGather",
                        mybir.AluOpType.bypass,
                        replica_groups=[[0, 1], [2, 3], [4, 5], [6, 7]],
                        ins=[in_t[:].opt()],
                        outs=[out_t[:].opt()],
                    )

                # ---------------- P1 ----------------
                with tc.tile_pool(name="p1a", bufs=1) as p1a:
                    wk8_sb = p1a.tile([128, 4, 2, D], F8, name="wk8", tag="wk8")
                    wq8_sb = p1a.tile([128, 4, 2, D], F8, name="wq8", tag="wq8")
                    wv_sb = p1a.tile([128, NDT, D], F16, name="wv", tag="wv")
                    xa = p1a.tile([128, NDT, S // 2], F16, name="xa", tag="xa")
                    xa8 = p1a.tile([128, 4, 2, S // 2], F8, name="xa8", tag="xa8")
                    xq8 = p1a.tile([128, 4, 2, 4 * CH], F8, name="xq8", tag="xq8")

                    # DMA issue is tiered: all in-flight DMAs share wire
                    # bandwidth, so only the critical-path wk/xa stream starts
                    # immediately (per-i granularity: compute starts on the
                    # first 256KB). Later streams are gated on compute
                    # sentinels and issued from the otherwise-idle vector
                    # engine so they cannot steal bandwidth early.
                    for h in range(2):
                        nc.gpsimd.dma_start(
                            wk8_sb[:, :, :, ts(h, 512)], wk8T[:, :, :, ts(h, 512)]
                        )
                        nc.sync.dma_start(
                            xa8[:, :, :, ts(h, 512)], xa8T[:, :, :, ts(h, 512)]
                        )
                    # wv / xa16 (V-pass inputs) stream right behind on the
                    # same queues; V starts ~15us after K
                    for h in range(4):
                        nc.gpsimd.dma_start(
                            wv_sb[:, ts(h, 2), :], wvT[:, ts(h, 2), :]
                        )
                        nc.sync.dma_start(
                            xa[:, ts(h, 2), :], xaT[:, ts(h, 2), :]
                        )

                    # ---- K projection over own half (kT staged at [:, :, 0:1024])
                    for tb in range(2):
                        for og in range(2):
                            ps4 = [
                                pspool.tile([128, 512], F32, name=f"at{j}", tag=f"at{j}", bufs=1)
                                for j in range(4)
                            ]
                            for i2 in range(4):
                                for j in range(4):
                                    nc.tensor.matmul(
                                        ps4[j][:],
                                        wk8_sb[:, i2, :, ts(og * 4 + j, 128)],
                                        xa8[:, i2, :, ts(tb, 512)],
                                        start=(i2 == 0), stop=(i2 == 3),
                                        perf_mode=DR,
                                    )
                            for j in range(4):
                                ot = og * 4 + j
                                nc.scalar.activation(
                                    kT8[:, ot // 2, ot % 2, ts(tb, 512)], ps4[j][:],
                                    AF.Identity, bias=bk_sb[:, ot : ot + 1],
                                    scale=1.0 / 64.0,
                                )
                            if tb == 0 and og == 0:
                                # tier-1 issue: scalar reaches here only after
                                # og0's drains, so later streams cannot steal
                                # wire bandwidth from the critical path
                                nc.scalar.dma_start(xq8[:], xq8T[:])
                        # stage this token block and fire its K gather
                        kin = ka_in if tb == 0 else kb_in
                        nc.sync.dma_start(kin[:], kT8[:, :, :, ts(tb, 512)])
                        pair_gather(kin, ka_out if tb == 0 else kb_out)

                    # ---- V projection over own half (token-major, 2 banks/tt)
                    for tb in range(2):
                        for tt in range(4):
                            ps2 = [
                                pspool.tile([128, 512], F32, name=f"mm{ob}", tag="mm", bufs=4)
                                for ob in range(2)
                            ]
                            for i in range(NDT):
                                for ob in range(2):
                                    nc.tensor.matmul(
                                        ps2[ob][:],
                                        xa[:, i, tb * 512 + tt * 128 : tb * 512 + (tt + 1) * 128],
                                        wv_sb[:, i, ts(ob, 512)],
                                        start=(i == 0), stop=(i == NDT - 1),
                                    )
                            for ob in range(2):
                                nc.scalar.activation(
                                    vt[:, tb * 4 + tt, ts(ob, 512)], ps2[ob][:], AF.Identity
                                )
                            if tb == 0 and tt == 0:
                                # tier-2 issue (see tier-1 note)
                                nc.scalar.dma_start(wq8_sb[:], wq8T[:])
                        # stage this V token block and fire its gather.
                        # vt[:, 0:8] doubles as the projection scratch; the
                        # readback overwrites all 16 slots in token order.
                        vin = va_in if tb == 0 else vb_in
                        nc.scalar.dma_start(vin[:], vt[:, ts(tb, 4), :])
                        pair_gather(vin, va_out if tb == 0 else vb_out)

                    # qpos broadcast (fills the gather window)
                    for i in range(4 * CH // 512):
                        bc_ps = pspool.tile([128, 512], F32, name="small", tag="at0", bufs=1)
                        nc.tensor.matmul(
                            bc_ps[:], ones_row[:], qpos_row[:, ts(i, 512)],
                            start=True, stop=True,
                        )
                        nc.scalar.activation(qposB[:, ts(i, 512)], bc_ps[:], AF.Identity)

                    # ---- Q projection: qb pair interleaved under one weight
                    for ot in range(NDT):
                        psq = [
                            pspool.tile([128, 512], F32, name=f"mmq{qb}", tag="mm", bufs=4)
                            for qb in range(2)
                        ]
                        for i2 in range(4):
                            for qb in range(2):
                                nc.tensor.matmul(
                                    psq[qb][:],
                                    wq8_sb[:, i2, :, ts(ot, 128)],
                                    xq8[:, i2, :, ts(qb, 512)],
                                    start=(i2 == 0), stop=(i2 == 3),
                                    perf_mode=DR,
                                )
                        for qb in range(2):
                            nc.scalar.activation(
                                qT8[qb][:, ot // 2, ot % 2, :], psq[qb][:],
                                AF.Identity, bias=bq_sb[:, ot : ot + 1],
                                scale=1.0 / 64.0,
                            )
                        if ot == 0:
                            # tier-3: wo is only needed in P3; issuing it this
                            # late keeps it off the K/V gather's wire window
                            for h in range(2):
                                nc.scalar.dma_start(
                                    wo_sb[:, ts(h, 4), :], woT[:, ts(h, 4), :]
                                )

                # gathered K^T / V readback in true token order, ordered by
                # first use in P2 (sync engine; waits ride on the collectives)
                for r in range(2):
                    nc.sync.dma_start(
                        kT8[:, :, :, r * 1024 + 0 : r * 1024 + 512], ka_out[r]
                    )
                    nc.sync.dma_start(
                        kT8[:, :, :, r * 1024 + 512 : r * 1024 + 1024], kb_out[r]
                    )
                for r in range(2):
                    nc.sync.dma_start(vt[:, r * 8 + 0 : r * 8 + 4, :], va_out[r])
                    nc.sync.dma_start(vt[:, r * 8 + 4 : r * 8 + 8, :], vb_out[r])

                # ---------------- P2: attention ----------------
                # Slots are processed as PAIRS sharing a 512-token q block:
                # the k-range both slots need runs at FD=512 (LDWEIGHTS fully
                # hidden), the hi-slot's excess k-tiles at FD=256.
                # pass A computes scores+exp+mask+denom for all slots first
                # (probs stay resident), so the V gather hides behind it.
                with tc.tile_pool(name="p2", bufs=1) as p2:
                    LOHI = [(NKT[0], NKT[1]), (NKT[2], NKT[3])]
                    pt_sh = [
                        [
                            p2.tile([128, 512], F16, name=f"pts{p}_{k}", tag=f"pts{p}_{k}", bufs=1)
                            for k in range(LOHI[p][0])
                        ]
                        for p in range(2)
                    ]
                    pt_ex = [
                        [
                            p2.tile([128, CH], F16, name=f"ptx{p}_{j}", tag=f"ptx{p}_{j}", bufs=1)
                            for j in range(LOHI[p][1] - LOHI[p][0])
                        ]
                        for p in range(2)
                    ]
                    recipB = [
                        p2.tile([128, CH], F32, name=f"recipB{sl}", tag=f"recipB{sl}", bufs=1)
                        for sl in range(NSLOT)
                    ]
                    for p in range(2):
                        lo, hi = LOHI[p]
                        # shared k-range: FD=512 over both slots
                        for k in range(lo):
                            ps = pspool.tile([128, 512], F32, name="mm", tag="mm", bufs=4)
                            for i2 in range(4):
                                nc.tensor.matmul(
                                    ps[:],
                                    kT8[:, i2, :, ts(k, 128)],
                                    qT8[p][:, i2, :, :],
                                    start=(i2 == 0), stop=(i2 == 3),
                                    perf_mode=DR,
                                )
                            nc.scalar.activation(
                                pt_sh[p][k][:], ps[:], AF.Exp, scale=1.0 / 32.0
                            )
                            if k >= lo - 4:
                                # mask applies to the lo slot's columns only
                                msk = p2.tile([128, CH], F16, name="msk", tag="msk", bufs=2)
                                nc.vector.tensor_scalar(
                                    out=msk[:],
                                    in0=qposB[:, ts(2 * p, CH)],
                                    scalar1=iota_sb[:, k : k + 1],
                                    scalar2=None,
                                    op0=ALU.is_ge,
                                )
                                nc.vector.tensor_tensor(
                                    out=pt_sh[p][k][:, 0:CH],
                                    in0=pt_sh[p][k][:, 0:CH], in1=msk[:],
                                    op=ALU.mult,
                                )
                        # excess k-tiles: hi slot only, FD=256, all masked
                        for j, k in enumerate(range(lo, hi)):
                            ps = pspool.tile([128, CH], F32, name="mm", tag="mm", bufs=4)
                            for i2 in range(4):
                                nc.tensor.matmul(
                                    ps[:],
                                    kT8[:, i2, :, ts(k, 128)],
                                    qT8[p][:, i2, :, CH : 2 * CH],
                                    start=(i2 == 0), stop=(i2 == 3),
                                    perf_mode=DR,
                                )
                            praw = p2.tile([128, CH], F16, name="praw", tag="praw", bufs=2)
                            nc.scalar.activation(
                                praw[:], ps[:], AF.Exp, scale=1.0 / 32.0
                            )
                            msk = p2.tile([128, CH], F16, name="msk", tag="msk", bufs=2)
                            nc.vector.tensor_scalar(
                                out=msk[:],
                                in0=qposB[:, ts(2 * p + 1, CH)],
                                scalar1=iota_sb[:, k : k + 1],
                                scalar2=None,
                                op0=ALU.is_ge,
                            )
                            nc.vector.tensor_tensor(
                                out=pt_ex[p][j][:], in0=praw[:], in1=msk[:],
                                op=ALU.mult,
                            )
                        # denominators per slot (FD=256 chains), then recip
                        for h_ in range(2):
                            sl = 2 * p + h_
                            dn_ps = pspool.tile([1, CH], F32, name="small", tag="at0", bufs=1)
                            nmm = LOHI[p][0] if h_ == 0 else hi
                            kk = 0
                            for k in range(lo):
                                nc.tensor.matmul(
                                    dn_ps[:], ones_col_bf[:],
                                    pt_sh[p][k][:, ts(h_, CH)],
                                    start=(kk == 0), stop=(kk == nmm - 1),
                                )
                                kk += 1
                                if h_ == 0 and kk == nmm:
                                    break
                            if h_ == 1:
                                for j in range(hi - lo):
                                    nc.tensor.matmul(
                                        dn_ps[:], ones_col_bf[:], pt_ex[p][j][:],
                                        start=(kk == 0), stop=(kk == nmm - 1),
                                    )
                                    kk += 1
                            dn_r = p2.tile([1, CH], F32R, name="dn_r", tag="dn_r", bufs=2)
                            nc.vector.tensor_copy(dn_r[:], dn_ps[:])
                            rb_ps = pspool.tile([128, CH], F32, name="small2", tag="at1", bufs=1)
                            nc.tensor.matmul(
                                rb_ps[:], ones_row[:], dn_r[:], start=True, stop=True
                            )
                            nc.vector.reciprocal(recipB[sl][:], rb_ps[:])

                    # pass B: attn^T = (P @ V)^T scaled by 1/denom
                    for p in range(2):
                        lo, hi = LOHI[p]
                        for half in range(2):
                            for d4 in range(4):
                                d_ = half * 4 + d4
                                aps = pspool.tile([128, 512], F32, name=f"at{d4}", tag=f"at{d4}", bufs=1)
                                for k in range(lo):
                                    nc.tensor.matmul(
                                        aps[:],
                                        vt[:, k, ts(d_, 128)],
                                        pt_sh[p][k][:],
                                        start=(k == 0), stop=False,
                                        skip_group_check=True,
                                    )
                                for j, k in enumerate(range(lo, hi)):
                                    nc.tensor.matmul(
                                        aps[:, CH : 2 * CH],
                                        vt[:, k, ts(d_, 128)],
                                        pt_ex[p][j][:],
                                        start=False, stop=(k == hi - 1),
                                        skip_group_check=True,
                                    )
                                if hi == lo:
                                    pass
                                for h_ in range(2):
                                    nc.vector.tensor_tensor(
                                        out=attnT[d_][p][:, ts(h_, CH)],
                                        in0=aps[:, ts(h_, CH)],
                                        in1=recipB[2 * p + h_][:],
                                        op=ALU.mult,
                                    )

                # ---------------- P3 + P4 ----------------
                with tc.tile_pool(name="p34", bufs=1) as p34:
                    outT = [
                        [p34.tile([128, 512], F16, name=f"oT{dt_}_{qb}", tag=f"oT{dt_}_{qb}") for qb in range(2)]
                        for dt_ in range(NDT)
                    ]
                    bfbc = p34.tile([128, 8, 512], F16, name="bfbc", tag="bfbc")
                    for mb in range(M // 512):
                        bc_ps = pspool.tile([128, 512], F32, name="small", tag="at0", bufs=1)
                        nc.tensor.matmul(
                            bc_ps[:], ones_row[:], bfr_sb[:, ts(mb, 512)],
                            start=True, stop=True,
                        )
                        nc.scalar.activation(bfbc[:, mb, :], bc_ps[:], AF.Identity)
                    # P3: qb pair interleaved under one Wo weight block
                    for ot in range(NDT):
                        pso = [
                            pspool.tile([128, 512], F32, name=f"mmo{qb}", tag="mm", bufs=4)
                            for qb in range(2)
                        ]
                        for i in range(NDT):
                            for qb in range(2):
                                nc.tensor.matmul(
                                    pso[qb][:],
                                    wo_sb[:, i, ts(ot, 128)],
                                    attnT[i][qb][:],
                                    start=(i == 0), stop=(i == NDT - 1),
                                )
                        for qb in range(2):
                            nc.scalar.activation(
                                outT[ot][qb][:], pso[qb][:], AF.Identity,
                                bias=bo2_sb[:, ot : ot + 1],
                            )

                    # P4: FFN + GELU. Stationary = outT token-blocks, moving
                    # = the full 512-wide wf block: LDWEIGHTS hides under the
                    # previous matmul (K-pass pattern, 8-bank rotation). The
                    # free-dim bias bf is pre-filled into PSUM by the vector
                    # engine; matmuls accumulate on top (start=False).
                    for mb in range(M // 512):
                        wfb = p34.tile([128, NDT, 512], F16, name="wfb", tag="wfb", bufs=2)
                        nc.gpsimd.dma_start(wfb[:], wfT[mb])
                        st = p34.tile([128, 8, 512], F16, name="ffstage", tag="ffstage", bufs=2)
                        ps8 = [
                            pspool.tile(
                                [128, 512], F32, name=f"ps8_{t8}",
                                tag=(f"at{t8}" if t8 < 4 else "mm"),
                                bufs=(1 if t8 < 4 else 4),
                            )
                            for t8 in range(8)
                        ]
                        for t8 in range(8):
                            qb, tb2 = divmod(t8, 4)
                            nc.vector.tensor_copy(ps8[t8][:], bfbc[:, mb, :])
                            for i in range(NDT):
                                nc.tensor.matmul(
                                    ps8[t8][:],
                                    outT[i][qb][:, ts(tb2, 128)],
                                    wfb[:, i, :],
                                    start=False, stop=(i == NDT - 1),
                                    skip_group_check=True,
                                )
                            nc.scalar.activation(st[:, t8, :], ps8[t8][:], AF.Gelu)
                            if t8 == 3:
                                nc.sync.dma_start(ffT[mb, :, 0:4, :], st[:, 0:4, :])
                        nc.sync.dma_start(ffT[mb, :, 4:8, :], st[:, 4:8, :])

    nc.compile()
    return nc


def _get_program():
    global _PROGRAM
    if _PROGRAM is None:
        _PROGRAM = _build_program()
    return _PROGRAM


def _owned_chunks(core):
    """The four 256-token chunk indices this core owns, in slot order."""
    if core % 2 == 0:
        return (0, 3, 4, 7)
    return (1, 2, 5, 6)


def _blocked(a):
    """[1024, W] -> [128, 8, W] with [p, i, c] = a[i*128+p, c]."""
    W = a.shape[1]
    return np.ascontiguousarray(a.reshape(8, 128, W).transpose(1, 0, 2))


def _pair8(a, scale=1.0):
    """[1024, W] -> [128, 4, 2, W] e4m3 with [p, i2, s, c] = scale*a[(2i2+s)*128+p, c]."""
    W = a.shape[1]
    t = (np.asarray(a, np.float32) * scale).reshape(4, 2, 128, W).transpose(2, 0, 1, 3)
    return np.ascontiguousarray(t).astype(_E4M3)


def _make_in_maps(x, Wq, bq, Wk, bk, Wv, bv, Wo, bo, Wf, bf):
    f32, f16 = np.float32, np.float16
    wq8T = _pair8(np.asarray(Wq.T, np.float32), 64.0)
    wk8T = _pair8(np.asarray(Wk.T, np.float32), 64.0)
    wvT = _blocked(np.asarray(Wv.T, dtype=f16))
    woT = _blocked(np.asarray(Wo.T, dtype=f16))
    # wfT[mb, p, i, c] = Wf.T[i*128+p, mb*512+c]
    wfT = np.ascontiguousarray(
        np.asarray(Wf.T, dtype=f16).reshape(8, 128, 8, 512).transpose(2, 1, 0, 3)
    )
    bo2 = (Wo.astype(np.float64) @ bv.astype(np.float64) + bo.astype(np.float64))
    bo2 = np.ascontiguousarray(bo2.astype(f32).reshape(D // 128, 128).T)
    bfT = np.ascontiguousarray(bf.reshape(M // 128, 128).T, dtype=f32)
    iota = (
        np.arange(128, dtype=f32)[:, None]
        + 128.0 * np.arange(S // 128, dtype=f32)[None, :]
    )
    shared = {
        "wq8T": wq8T, "wk8T": wk8T, "wvT": wvT, "woT": woT, "wfT": wfT,
        "bq": np.ascontiguousarray(bq.reshape(D // 128, 128).T, dtype=f32),
        "bk": np.ascontiguousarray(bk.reshape(D // 128, 128).T, dtype=f32),
        "bo2": bo2,
        "bfT": bfT,
        "bf_row": np.ascontiguousarray(bf[None, :].astype(f32)),
        "iota_kt": np.ascontiguousarray(iota),
    }
    in_maps = []
    for core in range(N_CORES):
        b = core // 2
        chunks = _owned_chunks(core)
        xTb = np.asarray(x[b].T, dtype=f16)  # [D, S]
        half = core % 2  # rank within the pair: rank0 owns tokens 0:S/2
        xown = xTb[:, half * (S // 2) : (half + 1) * (S // 2)]
        xaT = _blocked(xown)
        xa8T = _pair8(xown.astype(np.float32))
        xq8T = _pair8(
            np.concatenate(
                [xTb[:, c * CH : (c + 1) * CH] for c in chunks], axis=1
            ).astype(np.float32)
        )
        qp = np.concatenate(
            [np.arange(c * CH, (c + 1) * CH) for c in chunks]
        ).astype(f32)[None, :]
        in_maps.append(
            {**shared, "xaT": xaT, "xa8T": xa8T, "xq8T": xq8T,
             "qpos": np.ascontiguousarray(qp)}
        )
    return in_maps


def _run(inputs, trace=False, trace_cores=None, tmpdir=None):
    import sys

    if "/opt/trn_rl_repo" not in sys.path:
        sys.path.insert(0, "/opt/trn_rl_repo")
    from concourse.bass_utils import run_bass_kernel_spmd

    nc = _get_program()
    in_maps = _make_in_maps(**inputs)
    res = run_bass_kernel_spmd(
        nc, in_maps, list(range(N_CORES)), trace=trace, trace_cores=trace_cores,
        tmpdir=tmpdir,
    )
    out = np.empty((B, S, M), dtype=np.float32)
    for core in range(N_CORES):
        b = core // 2
        chunks = _owned_chunks(core)
        # ffT[mb, p, t8, c] = ff[(t8//4)*512 + (t8%4)*128 + p, mb*512 + c]
        raw = res.results[core]["ffT"].reshape(8, 128, 2, 4, 512)
        ff = np.ascontiguousarray(
            raw.transpose(2, 3, 1, 0, 4)
        ).reshape(4 * CH, M)
        for sl, c in enumerate(chunks):
            qb, qo = divmod(sl, 2)
            out[b, c * CH : (c + 1) * CH] = (
                ff[qb * 512 + qo * CH : qb * 512 + (qo + 1) * CH].astype(np.float32)
            )
    return out, res


def kernel(**inputs):
    out, _ = _run(inputs)
    return out
